# revision 19
# baseline (speedup 1.0000x reference)
"""Trainium2 Bass kernel for nn_CausalGemAttention.

Reference computation (B=2, T=2048, C=1024, H=16, d=64):
    qkv = x @ w_attn + b_attn ; q,k,v = split(qkv)
    p = sign(sign(p_param)+0.5) * clamp(|p_param|, 1e-4, 1e3)
    vc = clip(|v + 5|, 1e-10); z = p*ln(vc); zmax = max_T(z); v' = exp(z - zmax)
    att = causal_softmax(q k^T / sqrt(d)); mean = att @ v'
    y = exp((zmax + ln(mean)) / p) - 5 ; out = y @ w_proj + b_proj

Sharding: 8 cores = 2 (batch) x 4 (head groups of 4 heads / 256 channels).
Each core computes qkv for its head group (contraction over full C), local
attention, and a partial projection (w_proj rows of its channels); host sums
the 4 partials per batch and adds b_proj.

Matmul operands are bf16 with fp32 PSUM accumulation.  To keep bf16 rounding
out of the softmax average (the +5 shift amplifies relative error of the
mean ~8x), v' is centered per channel before the PV matmul:
    mean = num''/den + cmid   with   v'' = v' - cmid
cmid is carried in fp32 and re-added exactly.  When p == 1 (the shipped
configuration) the final transform reduces to y = exp(zmax)*mean - 5 and is
computed without any per-tile ln/exp; a general-p fallback path exists.
"""

import sys
sys.path.insert(0, "/opt/trn_rl_repo")

import numpy as np
import ml_dtypes

import concourse.bacc as bacc
import concourse.tile as tile
from concourse import mybir
from concourse.bass_utils import run_bass_kernel_spmd

F32 = mybir.dt.float32
F32R = mybir.dt.float32r
BF16 = mybir.dt.bfloat16
AF = mybir.ActivationFunctionType
ALU = mybir.AluOpType
AX = mybir.AxisListType

B, T, C, H, D = 2, 2048, 1024, 16, 64
P = 128
CL = 256            # channels per core (4 heads x 64)
KC = C // P         # 8 contraction chunks for qkv
NQ = T // 512       # 4 query blocks of 512
NK = T // P         # 16 key tiles of 128
SHIFT = 5.0
P_MIN, P_MAX, V_MIN = 1e-4, 1e3, 1e-10
SM_SCALE = 1.0 / 8.0  # 1/sqrt(64)

_CACHE = {}


def _build(fast_p1):
    nc = bacc.Bacc("TRN2", target_bir_lowering=False, debug=False)

    xt_d = nc.dram_tensor("xt", [C, T], BF16, kind="ExternalInput")
    wq_d = nc.dram_tensor("wq", [C, CL], BF16, kind="ExternalInput")
    wk_d = nc.dram_tensor("wk", [C, CL], BF16, kind="ExternalInput")
    wv_d = nc.dram_tensor("wv", [C, CL], BF16, kind="ExternalInput")
    wp_d = nc.dram_tensor("wp", [CL, C], BF16, kind="ExternalInput")
    # bps = [bq(2) | bk(2) | bv5(2) | pp(2)] per chunk, fp32
    bps_d = nc.dram_tensor("bps", [P, 8], F32, kind="ExternalInput")
    # cst = [masks(4x512) | ident(64) | onesc(16)] packed along free dim
    cst_d = nc.dram_tensor("cst", [P, 2128], BF16, kind="ExternalInput")
    or_d = nc.dram_tensor("onesr", [1, 64], F32R, kind="ExternalInput")
    out_d = nc.dram_tensor("out_p", [T, C], F32, kind="ExternalOutput")

    with tile.TileContext(nc) as tc:
        with (
            tc.tile_pool(name="consts", bufs=1) as cp,
            tc.tile_pool(name="qk", bufs=1) as qkp,
            tc.tile_pool(name="vy", bufs=1) as vyp,
        ):
            cst = cp.tile([P, 2128], BF16)
            onesr = cp.tile([1, 64], F32R)
            bps = cp.tile([P, 8], F32)
            nc.sync.dma_start(bps[:], bps_d[:])
            ident = cst[:, 2048:2112]
            bq_sb = bps[:, 0:2]
            bk_sb = bps[:, 2:4]
            bv5_sb = bps[:, 4:6]
            pp_sb = bps[:, 6:8]

            # p = sign(sign(pp)+0.5) * clamp(|pp|, P_MIN, P_MAX); ip = 1/p
            sgn = cp.tile([P, 2], F32)
            ab = cp.tile([P, 2], F32)
            p_sb = cp.tile([P, 2], F32)
            # allcp packs [ip | zmaxp | cmid | ezp | ecp5] x 2 chunks
            allcp = cp.tile([P, 5, 2], F32)
            ip_sb = allcp[:, 0, :]
            zmaxp = allcp[:, 1, :]
            cmid = allcp[:, 2, :]
            ezp = allcp[:, 3, :]
            ecp5 = allcp[:, 4, :]
            nc.scalar.activation(sgn[:], pp_sb[:], AF.Sign)
            nc.vector.tensor_scalar_add(sgn[:], sgn[:], 0.5)
            nc.scalar.activation(sgn[:], sgn[:], AF.Sign)
            nc.scalar.activation(ab[:], pp_sb[:], AF.Abs)
            nc.vector.tensor_scalar(ab[:], ab[:], float(P_MIN), float(P_MAX),
                                    ALU.max, ALU.min)
            nc.vector.tensor_tensor(p_sb[:], sgn[:], ab[:], ALU.mult)
            nc.vector.reciprocal(ip_sb[:], p_sb[:])

            negzmax = cp.tile([P, 2], F32)
            zmin_sb = cp.tile([P, 2], F32)
            # allh: per-head [64,1] base-0 views of allcp, [64, const, head]
            allh = cp.tile([64, 5, 4], F32)
            iph = allh[:, 0, :]
            zmh = allh[:, 1, :]
            cmh = allh[:, 2, :]
            eph = allh[:, 3, :]
            ech = allh[:, 4, :]

            qT = qkp.tile([P, 2, T], BF16)   # q^T: [c%128, c//128, t]
            kT = qkp.tile([P, 2, T], BF16)
            vnat = vyp.tile([P, 4, NK, 65], BF16)  # [tk%128, head, tk//128, d|1]
            yT = vyp.tile([P, 2, T], BF16)
            wp_sb = vyp.tile([P, 2, C], BF16)
            # padded per-head q/k (K=128 with zero rows 64:127): K=64 matmuls
            # measure ~1.5x slower than K=128 on the PE, so pad instead.
            qTp = vyp.tile([P, 4, T], BF16)
            kTp = vyp.tile([P, 4, T], BF16)

            # ---------------- Phase A: qkv;  B: v transform + transposes -----
            with (
                tc.tile_pool(name="pA", bufs=1) as pA,
                tc.tile_pool(name="pB", bufs=1) as pB,
                tc.tile_pool(name="psA", bufs=6, space="PSUM") as psA,
            ):
                xt_sb = pA.tile([P, KC, T], BF16)
                wq_sb = pA.tile([P, KC, CL], BF16)
                wk_sb = pA.tile([P, KC, CL], BF16)
                wv_sb = pA.tile([P, KC, CL], BF16)
                for wsb_, wd_ in ((wv_sb, wv_d), (wq_sb, wq_d), (wk_sb, wk_d)):
                    wr = wd_[:].rearrange("(a p) m -> p a m", p=P)
                    nc.sync.dma_start(wsb_[:, 0:2, :], wr[:, 0:2, :])
                    nc.sync.dma_start(wsb_[:, 2:KC, :], wr[:, 2:KC, :])
                for kc in range(KC):
                    for hf in range(2):
                        nc.sync.dma_start(
                            xt_sb[:, kc, hf * 1024:(hf + 1) * 1024],
                            xt_d[kc * P:(kc + 1) * P, hf * 1024:(hf + 1) * 1024])
                nc.sync.dma_start(cst[:], cst_d[:])
                nc.sync.dma_start(onesr[:], or_d[:])
                nc.sync.dma_start(wp_sb[:],
                                  wp_d[:].rearrange("(c p) n -> p c n", p=P))
                nc.vector.memset(qTp[64:128, :, :], 0.0)
                nc.vector.memset(kTp[64:128, :, :], 0.0)

                vT = pB.tile([P, 2, T], F32)
                vpT = pB.tile([P, 2, T], BF16)

                def qkv_group(wsb, kind, m, nt):
                    ps = psA.tile([P, 512], F32, tag="ev", name="ev")
                    for kc in range(KC):
                        nc.tensor.matmul(
                            ps[:],
                            wsb[:, kc, m * P:(m + 1) * P],
                            xt_sb[:, kc, nt * 512:(nt + 1) * 512],
                            start=(kc == 0), stop=(kc == KC - 1),
                        )
                    tsl = slice(nt * 512, (nt + 1) * 512)
                    if kind == "q":
                        nc.vector.tensor_scalar_add(
                            qT[:, m, tsl], ps[:], bq_sb[:, m:m + 1])
                    elif kind == "k":
                        nc.vector.tensor_scalar_add(
                            kT[:, m, tsl], ps[:], bk_sb[:, m:m + 1])
                    else:
                        # |v + b + SHIFT| directly out of PSUM
                        nc.scalar.activation(
                            vT[:, m, tsl], ps[:], AF.Abs,
                            bias=bv5_sb[:, m:m + 1])

                # v first so its transform overlaps the q/k matmuls
                for m in range(2):
                    for nt in range(NQ):
                        qkv_group(wv_sb, "v", m, nt)

                # transform: z = p*ln(clip(vc)); zmax/zmin; v'' = e^(z-zmax)-cmid
                for m in range(2):
                    nc.vector.tensor_scalar_max(vT[:, m, :], vT[:, m, :],
                                                float(V_MIN))
                    nc.scalar.activation(vT[:, m, :], vT[:, m, :], AF.Ln)
                for m in range(2):
                    nc.vector.tensor_scalar_mul(vT[:, m, :], vT[:, m, :],
                                                p_sb[:, m:m + 1])
                    nc.vector.tensor_reduce(negzmax[:, m:m + 1], vT[:, m, :], AX.X,
                                            op=ALU.max, negate=True)
                    nc.vector.tensor_reduce(zmin_sb[:, m:m + 1], vT[:, m, :], AX.X,
                                            op=ALU.min)
                    nc.vector.scalar_tensor_tensor(
                        zmaxp[:, m:m + 1], negzmax[:, m:m + 1], -1.0,
                        ip_sb[:, m:m + 1], ALU.mult, ALU.mult)
                for m in range(2):
                    # cmid = 0.5*(1 + exp(zmin - zmax))
                    nc.scalar.activation(cmid[:, m:m + 1], zmin_sb[:, m:m + 1],
                                         AF.Exp, bias=negzmax[:, m:m + 1])
                    nc.vector.tensor_scalar(cmid[:, m:m + 1], cmid[:, m:m + 1],
                                            1.0, 0.5, ALU.add, ALU.mult)
                    # ezp = exp(zmax); ecp5 = ezp*cmid - 5
                    nc.scalar.activation(ezp[:, m:m + 1], negzmax[:, m:m + 1],
                                         AF.Exp, scale=-1.0)
                    nc.vector.scalar_tensor_tensor(
                        ecp5[:, m:m + 1], ezp[:, m:m + 1], 0.0,
                        cmid[:, m:m + 1], ALU.bypass, ALU.mult)
                    nc.vector.tensor_scalar_add(ecp5[:, m:m + 1], ecp5[:, m:m + 1],
                                                -SHIFT)
                    # v' (fp32, in place over z) then centered bf16 copy
                    nc.scalar.activation(vT[:, m, :], vT[:, m, :], AF.Exp,
                                         bias=negzmax[:, m:m + 1])
                    nc.vector.tensor_scalar_sub(vpT[:, m, :], vT[:, m, :],
                                                cmid[:, m:m + 1])

                # q/k matmuls (PE work that overlaps the v transform above)
                for m in range(2):
                    for nt in range(NQ):
                        qkv_group(wq_sb, "q", m, nt)
                        qkv_group(wk_sb, "k", m, nt)
                    for h in (2 * m, 2 * m + 1):
                        base = 64 * (h % 2)
                        nc.gpsimd.dma_start(qTp[0:64, h, :],
                                            qT[base:base + 64, m, :])
                        nc.gpsimd.dma_start(kTp[0:64, h, :],
                                            kT[base:base + 64, m, :])

                # per-head constants at partition base 0: heads (0,2) from
                # chunk rows 0:64, heads (1,3) from rows 64:128
                nc.sync.dma_start(allh[:, :, 0::2], allcp[0:64, :, :])
                nc.sync.dma_start(allh[:, :, 1::2], allcp[64:128, :, :])
                for h in range(4):
                    nc.sync.dma_start(vnat[:, h, :, 64], cst_d[:, 2112:2128])

                # transpose v''^T [d, tk] -> vnat [tk, d], 8 k-tiles per bank
                for h in range(4):
                    base, ch = 64 * (h % 2), h // 2
                    for half in range(2):
                        trp = psA.tile([P, 512], BF16, tag="tr", name="trp", bufs=2)
                        for j in range(8):
                            kt = half * 8 + j
                            nc.tensor.transpose(
                                trp[:, j * 64:(j + 1) * 64],
                                vpT[base:base + 64, ch, kt * P:(kt + 1) * P],
                                cst[base:base + 64, 2048:2112],
                            )
                        nc.vector.tensor_copy(
                            vnat[:, h, half * 8:(half + 1) * 8, 0:64],
                            trp[:].rearrange("p (a b) -> p a b", a=8),
                        )

            # ---------------- Phase C: attention ------------------------------
            with (
                tc.tile_pool(name="att", bufs=5) as att,
                tc.tile_pool(name="small", bufs=3) as sm,
                tc.tile_pool(name="outp", bufs=3) as op_,
                tc.tile_pool(name="psS", bufs=2, space="PSUM") as psS,
                tc.tile_pool(name="psV", bufs=2, space="PSUM") as psV,
                tc.tile_pool(name="psX", bufs=2, space="PSUM") as psX,
            ):
                pending = []   # deferred post-chains (emitted mid next q-block)

                def proj_group(tq):
                    po = op_.tile([P, C], F32, tag="po", name="po")
                    for nh in range(2):
                        pj = psX.tile([P, 512], F32, tag="x", name="pj")
                        for c in range(2):
                            nc.tensor.matmul(
                                pj[:],
                                yT[:, c, tq * P:(tq + 1) * P],
                                wp_sb[:, c, nh * 512:(nh + 1) * 512],
                                start=(c == 0), stop=(c == 1),
                            )
                        if nh == 0:
                            nc.scalar.activation(po[:, 0:512], pj[:], AF.Copy)
                        else:
                            nc.vector.tensor_copy(po[:, 512:1024], pj[:])
                    nc.gpsimd.dma_start(out_d[tq * P:(tq + 1) * P, :], po[:])

                def post_chain(pv_t, h):
                    # mean'' = num''/den ; y = ezp*mean'' + (ezp*cmid - 5)
                    dcp = sm.tile([1, 512], F32, tag="dcp", name="dcp")
                    nc.vector.tensor_copy(dcp[:], pv_t[64:65, :])
                    rdf = sm.tile([1, 512], F32, tag="rdf", name="rdf")
                    nc.vector.reciprocal_approx_fast(rdf[:], dcp[:])
                    rd = sm.tile([1, 512], F32R, tag="rd", name="rd")
                    nc.vector.tensor_copy(rd[:], rdf[:])
                    bc = psX.tile([64, 512], F32, tag="x", name="bc",
                                  padded_shape=[P, 512])
                    nc.tensor.matmul(bc[:], onesr[:], rd[:], start=True, stop=True)
                    me = sm.tile([64, 512], F32, tag="me", name="me")
                    nc.vector.tensor_copy(me[:], pv_t[0:64, :])
                    yh = sm.tile([64, 512], BF16, tag="yh", name="yh")
                    if fast_p1:
                        y1 = sm.tile([64, 512], F32, tag="y1", name="y1")
                        nc.vector.scalar_tensor_tensor(
                            y1[:], me[:], eph[:, h:h + 1], bc[:],
                            ALU.mult, ALU.mult)
                        nc.vector.tensor_scalar_add(yh[:], y1[:],
                                                    ech[:, h:h + 1])
                    else:
                        nc.vector.tensor_tensor(me[:], me[:], bc[:], ALU.mult)
                        nc.vector.tensor_scalar_add(me[:], me[:],
                                                    cmh[:, h:h + 1])
                        nc.scalar.activation(me[:], me[:], AF.Ln)
                        nc.scalar.activation(yh[:], me[:], AF.Exp,
                                             scale=iph[:, h:h + 1],
                                             bias=zmh[:, h:h + 1])
                        nc.vector.tensor_scalar_add(yh[:], yh[:], -SHIFT)
                    base, ch = 64 * (h % 2), h // 2
                    qi_ = post_chain_qi[0]
                    nc.gpsimd.dma_start(
                        yT[base:base + 64, ch, qi_ * 512:(qi_ + 1) * 512], yh[:])

                post_chain_qi = [0]

                for hp in range(2):
                    h0, h1 = 2 * hp, 2 * hp + 1
                    ch = hp
                    for qi in range(NQ):
                        npair = 2 * (qi + 1)       # kt pairs (kt = 2a, 2a+1)
                        qsl = slice(qi * 512, (qi + 1) * 512)
                        pv = [psV.tile([65, 512], F32, tag="pv", name=f"pv{_i}")
                              for _i in range(2)]
                        prev = None
                        for a in range(npair):
                            ptile = []
                            # scores: 2 heads row-packed on the PE array
                            s_ps = [psS.tile([P, 1024], F32, tag="s",
                                             name=f"s{_i}") for _i in range(2)]
                            for half in range(2):
                                kt = 2 * a + half
                                ksl = slice(kt * P, (kt + 1) * P)
                                for i, h in enumerate((h0, h1)):
                                    nc.tensor.matmul(
                                        s_ps[i][:, half * 512:(half + 1) * 512],
                                        kTp[:, h, ksl],
                                        qTp[:, h, qsl],
                                        start=True, stop=True,
                                        skip_group_check=True,
                                    )
                            for i in range(2):
                                pt = att.tile([P, 1024], BF16, tag="pT",
                                              name="pt")
                                nc.scalar.activation(pt[:], s_ps[i][:], AF.Exp,
                                                     scale=SM_SCALE)
                                j0 = 2 * a - 4 * qi
                                if j0 >= 0:   # diagonal band: mask pair
                                    nc.vector.tensor_mul(
                                        pt[:],
                                        pt[:],
                                        cst[:, j0 * 512:(j0 + 2) * 512])
                                ptile.append(pt)
                            if prev is not None:
                                pa_, pp0, pp1 = prev
                                for i, ppt in enumerate((pp0, pp1)):
                                    for half in range(2):
                                        kt = 2 * pa_ + half
                                        nc.tensor.matmul(
                                            pv[i][:],
                                            vnat[:, (h0, h1)[i], kt, :],
                                            ppt[:, half * 512:(half + 1) * 512],
                                            start=(kt == 0), stop=False,
                                            skip_group_check=True,
                                        )
                            if a == 1 and pending:
                                for fn in pending:
                                    fn()
                                pending.clear()
                                if hp == 1 and qi > 0:
                                    for tq in range(4 * (qi - 1), 4 * qi):
                                        proj_group(tq)
                            prev = (a, ptile[0], ptile[1])
                        pa_, pp0, pp1 = prev
                        for i, ppt in enumerate((pp0, pp1)):
                            for half in range(2):
                                kt = 2 * pa_ + half
                                nc.tensor.matmul(
                                    pv[i][:],
                                    vnat[:, (h0, h1)[i], kt, :],
                                    ppt[:, half * 512:(half + 1) * 512],
                                    start=(kt == 0), stop=(half == 1),
                                    skip_group_check=True,
                                )

                        def mk(pv_t, h, qi):
                            def fn():
                                post_chain_qi[0] = qi
                                post_chain(pv_t, h)
                            return fn
                        pending.append(mk(pv[0], h0, qi))
                        pending.append(mk(pv[1], h1, qi))
                for fn in pending:
                    fn()
                pending.clear()
                for tq in range(12, 16):
                    proj_group(tq)


    nc.finalize()
    return nc


def _host_inputs(x, w_attn, b_attn, w_proj, p_param):
    """Build the 8 per-core input dicts."""
    bf16 = ml_dtypes.bfloat16
    ident = np.concatenate([np.eye(64, dtype=np.float32)] * 2, axis=0)
    xx = np.arange(P, dtype=np.int64)[:, None]
    yy = np.arange(512, dtype=np.int64)[None, :]
    masks = np.concatenate(
        [(yy - xx - P * j >= 0).astype(np.float32) for j in range(4)], axis=1)
    onesc = np.ones((P, NK), dtype=np.float32)
    cst = np.concatenate([masks, ident, onesc], axis=1).astype(bf16)
    onesr = np.ones((1, 64), dtype=np.float32)

    xts = [np.ascontiguousarray(x[b].T).astype(bf16) for b in range(B)]
    in_maps = []
    for core in range(8):
        b, hg = divmod(core, 4)
        cs = slice(hg * CL, (hg + 1) * CL)
        csC = slice(C + hg * CL, C + (hg + 1) * CL)
        cs2C = slice(2 * C + hg * CL, 2 * C + (hg + 1) * CL)
        in_maps.append({
            "xt": xts[b],
            "wq": np.ascontiguousarray(w_attn[:, cs]).astype(bf16),
            "wk": np.ascontiguousarray(w_attn[:, csC]).astype(bf16),
            "wv": np.ascontiguousarray(w_attn[:, cs2C]).astype(bf16),
            "wp": np.ascontiguousarray(w_proj[cs, :]).astype(bf16),
            "bps": np.ascontiguousarray(np.concatenate([
                b_attn[cs].reshape(2, P).T,
                b_attn[csC].reshape(2, P).T,
                (b_attn[cs2C] + SHIFT).reshape(2, P).T,
                p_param[cs].reshape(2, P).T,
            ], axis=1).astype(np.float32)),
            "cst": cst,
            "onesr": onesr,
        })
    return in_maps


def kernel(x, w_attn, b_attn, w_proj, b_proj, p_param, _trace=False):
    x = np.asarray(x, dtype=np.float32)
    w_attn = np.asarray(w_attn, dtype=np.float32)
    b_attn = np.asarray(b_attn, dtype=np.float32)
    w_proj = np.asarray(w_proj, dtype=np.float32)
    b_proj = np.asarray(b_proj, dtype=np.float32)
    p_param = np.asarray(p_param, dtype=np.float32)

    # p == 1 admits a cheaper final transform (no per-tile ln/exp)
    p_eff = np.sign(np.sign(p_param) + 0.5) * np.clip(np.abs(p_param),
                                                      P_MIN, P_MAX)
    fast_p1 = bool(np.all(p_eff == 1.0))

    key = ("nc", fast_p1)
    if key not in _CACHE:
        _CACHE[key] = _build(fast_p1)
    nc = _CACHE[key]

    in_maps = _host_inputs(x, w_attn, b_attn, w_proj, p_param)
    res = run_bass_kernel_spmd(nc, in_maps, core_ids=list(range(8)),
                               trace=_trace)
    _CACHE["last_result"] = res

    out = np.zeros((B, T, C), dtype=np.float32)
    for core in range(8):
        b = core // 4
        out[b] += res.results[core]["out_p"]
    out += b_proj[None, None, :]
    return out


if __name__ == "__main__":
    rng = np.random.default_rng(0)
    ins = {
        "x": rng.standard_normal((B, T, C), dtype=np.float32),
        "w_attn": (rng.standard_normal((C, 3 * C), dtype=np.float32) * 0.02),
        "b_attn": np.zeros(3 * C, np.float32),
        "w_proj": (rng.standard_normal((C, C), dtype=np.float32) * 0.02),
        "b_proj": np.zeros(C, np.float32),
        "p_param": np.ones(C, np.float32),
    }
    out = kernel(**ins)
    print("ran, out shape", out.shape, "finite:", np.isfinite(out).all())


# revision 20
# speedup vs baseline: 1.1820x; 1.1820x over previous
"""Trainium2 Bass kernel for nn_CausalGemAttention.

Reference computation (B=2, T=2048, C=1024, H=16, d=64):
    qkv = x @ w_attn + b_attn ; q,k,v = split(qkv)
    p = sign(sign(p_param)+0.5) * clamp(|p_param|, 1e-4, 1e3)
    vc = clip(|v + 5|, 1e-10); z = p*ln(vc); zmax = max_T(z); v' = exp(z - zmax)
    att = causal_softmax(q k^T / sqrt(d)); mean = att @ v'
    y = exp((zmax + ln(mean)) / p) - 5 ; out = y @ w_proj + b_proj

Sharding: 8 cores = 2 (batch) x 4 (head groups of 4 heads / 256 channels).
Each core computes qkv for its head group (contraction over full C), local
attention, and a partial projection (w_proj rows of its channels); host sums
the 4 partials per batch and adds b_proj.

Matmul operands are bf16 with fp32 PSUM accumulation.  To keep bf16 rounding
out of the softmax average (the +5 shift amplifies relative error of the
mean ~8x), v' is centered per channel before the PV matmul:
    mean = num''/den + cmid   with   v'' = v' - cmid
cmid is carried in fp32 and re-added exactly.  When p == 1 (the shipped
configuration) the final transform reduces to y = exp(zmax)*mean - 5 and is
computed without any per-tile ln/exp; a general-p fallback path exists.
"""

import sys
sys.path.insert(0, "/opt/trn_rl_repo")

import numpy as np
import ml_dtypes

import concourse.bacc as bacc
import concourse.tile as tile
from concourse import mybir
from concourse.bass_utils import run_bass_kernel_spmd

F32 = mybir.dt.float32
F32R = mybir.dt.float32r
BF16 = mybir.dt.bfloat16
AF = mybir.ActivationFunctionType
ALU = mybir.AluOpType
AX = mybir.AxisListType

B, T, C, H, D = 2, 2048, 1024, 16, 64
P = 128
CL = 256            # channels per core (4 heads x 64)
KC = C // P         # 8 contraction chunks for qkv
NQ = T // 512       # 4 query blocks of 512
NK = T // P         # 16 key tiles of 128
SHIFT = 5.0
P_MIN, P_MAX, V_MIN = 1e-4, 1e3, 1e-10
SM_SCALE = 1.0 / 8.0  # 1/sqrt(64)

_CACHE = {}


def _build(fast_p1):
    nc = bacc.Bacc("TRN2", target_bir_lowering=False, debug=False)

    xt_d = nc.dram_tensor("xt", [C, T], BF16, kind="ExternalInput")
    wq_d = nc.dram_tensor("wq", [C, CL], BF16, kind="ExternalInput")
    wk_d = nc.dram_tensor("wk", [C, CL], BF16, kind="ExternalInput")
    wv_d = nc.dram_tensor("wv", [C, CL], BF16, kind="ExternalInput")
    wp_d = nc.dram_tensor("wp", [CL, C], BF16, kind="ExternalInput")
    # bps = [bq(2) | bk(2) | bv5(2) | pp(2)] per chunk, fp32
    bps_d = nc.dram_tensor("bps", [P, 8], F32, kind="ExternalInput")
    # cst = [masks(4x512) | ident(64) | onesc(16)] packed along free dim
    cst_d = nc.dram_tensor("cst", [P, 2128], BF16, kind="ExternalInput")
    or_d = nc.dram_tensor("onesr", [1, 64], F32R, kind="ExternalInput")
    out_d = nc.dram_tensor("out_p", [T, C], F32, kind="ExternalOutput")

    with tile.TileContext(nc) as tc:
        with (
            tc.tile_pool(name="consts", bufs=1) as cp,
            tc.tile_pool(name="qk", bufs=1) as qkp,
            tc.tile_pool(name="vy", bufs=1) as vyp,
        ):
            cst = cp.tile([P, 2128], BF16)
            onesr = cp.tile([1, 64], F32R)
            bps = cp.tile([P, 8], F32)
            nc.sync.dma_start(bps[:], bps_d[:])
            ident = cst[:, 2048:2112]
            bq_sb = bps[:, 0:2]
            bk_sb = bps[:, 2:4]
            bv5_sb = bps[:, 4:6]
            pp_sb = bps[:, 6:8]

            # p = sign(sign(pp)+0.5) * clamp(|pp|, P_MIN, P_MAX); ip = 1/p
            sgn = cp.tile([P, 2], F32)
            ab = cp.tile([P, 2], F32)
            p_sb = cp.tile([P, 2], F32)
            # allcp packs [ip | zmaxp | cmid | ezp | ecp5] x 2 chunks
            allcp = cp.tile([P, 5, 2], F32)
            ip_sb = allcp[:, 0, :]
            zmaxp = allcp[:, 1, :]
            cmid = allcp[:, 2, :]
            ezp = allcp[:, 3, :]
            ecp5 = allcp[:, 4, :]
            nc.scalar.activation(sgn[:], pp_sb[:], AF.Sign)
            nc.vector.tensor_scalar_add(sgn[:], sgn[:], 0.5)
            nc.scalar.activation(sgn[:], sgn[:], AF.Sign)
            nc.scalar.activation(ab[:], pp_sb[:], AF.Abs)
            nc.vector.tensor_scalar(ab[:], ab[:], float(P_MIN), float(P_MAX),
                                    ALU.max, ALU.min)
            nc.vector.tensor_tensor(p_sb[:], sgn[:], ab[:], ALU.mult)
            nc.vector.reciprocal(ip_sb[:], p_sb[:])

            negzmax = cp.tile([P, 2], F32)
            zmin_sb = cp.tile([P, 2], F32)
            # allh: per-head [64,1] base-0 views of allcp, [64, const, head]
            allh = cp.tile([64, 5, 4], F32)
            iph = allh[:, 0, :]
            zmh = allh[:, 1, :]
            cmh = allh[:, 2, :]
            eph = allh[:, 3, :]
            ech = allh[:, 4, :]

            qT = qkp.tile([P, 2, T], BF16)   # q^T: [c%128, c//128, t]
            kT = qkp.tile([P, 2, T], BF16)
            vnat = vyp.tile([P, 4, NK, 65], BF16)  # [tk%128, head, tk//128, d|1]
            yT = vyp.tile([P, 2, T], BF16)
            wp_sb = vyp.tile([P, 2, C], BF16)
            # padded per-head q/k (K=128 with zero rows 64:127): K=64 matmuls
            # measure ~1.5x slower than K=128 on the PE, so pad instead.
            qTp = vyp.tile([P, 4, T], BF16)
            kTp = vyp.tile([P, 4, T], BF16)

            # ---------------- Phase A: qkv;  B: v transform + transposes -----
            with (
                tc.tile_pool(name="pA", bufs=1) as pA,
                tc.tile_pool(name="pB", bufs=1) as pB,
                tc.tile_pool(name="psA", bufs=6, space="PSUM") as psA,
            ):
                xt_sb = pA.tile([P, KC, T], BF16)
                wq_sb = pA.tile([P, KC, CL], BF16)
                wk_sb = pA.tile([P, KC, CL], BF16)
                wv_sb = pA.tile([P, KC, CL], BF16)
                nc.sync.dma_start(wv_sb[:], wv_d[:].rearrange("(a p) m -> p a m", p=P))
                nc.sync.dma_start(wq_sb[:], wq_d[:].rearrange("(a p) m -> p a m", p=P))
                nc.sync.dma_start(wk_sb[:], wk_d[:].rearrange("(a p) m -> p a m", p=P))
                for kc in range(KC):
                    for hf in range(2):
                        nc.sync.dma_start(
                            xt_sb[:, kc, hf * 1024:(hf + 1) * 1024],
                            xt_d[kc * P:(kc + 1) * P, hf * 1024:(hf + 1) * 1024])
                nc.sync.dma_start(cst[:], cst_d[:])
                nc.sync.dma_start(onesr[:], or_d[:])
                nc.sync.dma_start(wp_sb[:],
                                  wp_d[:].rearrange("(c p) n -> p c n", p=P))
                nc.vector.memset(qTp[64:128, :, :], 0.0)
                nc.vector.memset(kTp[64:128, :, :], 0.0)

                vT = pB.tile([P, 2, T], F32)
                vpT = pB.tile([P, 2, T], BF16)

                def qkv_group(wsb, kind, m, nt):
                    ps = psA.tile([P, 512], F32, tag="ev", name="ev")
                    for kc in range(KC):
                        nc.tensor.matmul(
                            ps[:],
                            wsb[:, kc, m * P:(m + 1) * P],
                            xt_sb[:, kc, nt * 512:(nt + 1) * 512],
                            start=(kc == 0), stop=(kc == KC - 1),
                        )
                    tsl = slice(nt * 512, (nt + 1) * 512)
                    if kind == "q":
                        nc.vector.tensor_scalar_add(
                            qT[:, m, tsl], ps[:], bq_sb[:, m:m + 1])
                    elif kind == "k":
                        nc.vector.tensor_scalar_add(
                            kT[:, m, tsl], ps[:], bk_sb[:, m:m + 1])
                    else:
                        # |v + b + SHIFT| directly out of PSUM
                        nc.scalar.activation(
                            vT[:, m, tsl], ps[:], AF.Abs,
                            bias=bv5_sb[:, m:m + 1])

                # v first so its transform overlaps the q/k matmuls
                for m in range(2):
                    for nt in range(NQ):
                        qkv_group(wv_sb, "v", m, nt)

                # transform: z = p*ln(clip(vc)); zmax/zmin; v'' = e^(z-zmax)-cmid
                for m in range(2):
                    nc.vector.tensor_scalar_max(vT[:, m, :], vT[:, m, :],
                                                float(V_MIN))
                    nc.scalar.activation(vT[:, m, :], vT[:, m, :], AF.Ln)
                for m in range(2):
                    nc.vector.tensor_scalar_mul(vT[:, m, :], vT[:, m, :],
                                                p_sb[:, m:m + 1])
                    nc.vector.tensor_reduce(negzmax[:, m:m + 1], vT[:, m, :], AX.X,
                                            op=ALU.max, negate=True)
                    nc.vector.tensor_reduce(zmin_sb[:, m:m + 1], vT[:, m, :], AX.X,
                                            op=ALU.min)
                    nc.vector.scalar_tensor_tensor(
                        zmaxp[:, m:m + 1], negzmax[:, m:m + 1], -1.0,
                        ip_sb[:, m:m + 1], ALU.mult, ALU.mult)
                for m in range(2):
                    # cmid = 0.5*(1 + exp(zmin - zmax))
                    nc.scalar.activation(cmid[:, m:m + 1], zmin_sb[:, m:m + 1],
                                         AF.Exp, bias=negzmax[:, m:m + 1])
                    nc.vector.tensor_scalar(cmid[:, m:m + 1], cmid[:, m:m + 1],
                                            1.0, 0.5, ALU.add, ALU.mult)
                    # ezp = exp(zmax); ecp5 = ezp*cmid - 5
                    nc.scalar.activation(ezp[:, m:m + 1], negzmax[:, m:m + 1],
                                         AF.Exp, scale=-1.0)
                    nc.vector.scalar_tensor_tensor(
                        ecp5[:, m:m + 1], ezp[:, m:m + 1], 0.0,
                        cmid[:, m:m + 1], ALU.bypass, ALU.mult)
                    nc.vector.tensor_scalar_add(ecp5[:, m:m + 1], ecp5[:, m:m + 1],
                                                -SHIFT)
                    # v' (fp32, in place over z) then centered bf16 copy
                    nc.scalar.activation(vT[:, m, :], vT[:, m, :], AF.Exp,
                                         bias=negzmax[:, m:m + 1])
                    nc.vector.tensor_scalar_sub(vpT[:, m, :], vT[:, m, :],
                                                cmid[:, m:m + 1])

                # q/k matmuls (PE work that overlaps the v transform above)
                for m in range(2):
                    for nt in range(NQ):
                        qkv_group(wq_sb, "q", m, nt)
                        qkv_group(wk_sb, "k", m, nt)
                    for h in (2 * m, 2 * m + 1):
                        base = 64 * (h % 2)
                        nc.gpsimd.dma_start(qTp[0:64, h, :],
                                            qT[base:base + 64, m, :])
                        nc.gpsimd.dma_start(kTp[0:64, h, :],
                                            kT[base:base + 64, m, :])

                # per-head constants at partition base 0: heads (0,2) from
                # chunk rows 0:64, heads (1,3) from rows 64:128
                nc.sync.dma_start(allh[:, :, 0::2], allcp[0:64, :, :])
                nc.sync.dma_start(allh[:, :, 1::2], allcp[64:128, :, :])
                for h in range(4):
                    nc.sync.dma_start(vnat[:, h, :, 64], cst_d[:, 2112:2128])

                # transpose v''^T [d, tk] -> vnat [tk, d], 8 k-tiles per bank
                for h in range(4):
                    base, ch = 64 * (h % 2), h // 2
                    for half in range(2):
                        trp = psA.tile([P, 512], BF16, tag="tr", name="trp", bufs=2)
                        for j in range(8):
                            kt = half * 8 + j
                            nc.tensor.transpose(
                                trp[:, j * 64:(j + 1) * 64],
                                vpT[base:base + 64, ch, kt * P:(kt + 1) * P],
                                cst[base:base + 64, 2048:2112],
                            )
                        nc.vector.tensor_copy(
                            vnat[:, h, half * 8:(half + 1) * 8, 0:64],
                            trp[:].rearrange("p (a b) -> p a b", a=8),
                        )

            # ---------------- Phase C: attention ------------------------------
            with (
                tc.tile_pool(name="att", bufs=5) as att,
                tc.tile_pool(name="small", bufs=3) as sm,
                tc.tile_pool(name="outp", bufs=3) as op_,
                tc.tile_pool(name="psS", bufs=2, space="PSUM") as psS,
                tc.tile_pool(name="psV", bufs=2, space="PSUM") as psV,
                tc.tile_pool(name="psX", bufs=2, space="PSUM") as psX,
            ):
                pending = []   # deferred post-chains (emitted mid next q-block)

                def proj_group(tq):
                    po = op_.tile([P, C], F32, tag="po", name="po")
                    for nh in range(2):
                        pj = psX.tile([P, 512], F32, tag="x", name="pj")
                        for c in range(2):
                            nc.tensor.matmul(
                                pj[:],
                                yT[:, c, tq * P:(tq + 1) * P],
                                wp_sb[:, c, nh * 512:(nh + 1) * 512],
                                start=(c == 0), stop=(c == 1),
                            )
                        if nh == 0:
                            nc.scalar.activation(po[:, 0:512], pj[:], AF.Copy)
                        else:
                            nc.vector.tensor_copy(po[:, 512:1024], pj[:])
                    nc.gpsimd.dma_start(out_d[tq * P:(tq + 1) * P, :], po[:])

                def post_chain(pv_t, h):
                    # mean'' = num''/den ; y = ezp*mean'' + (ezp*cmid - 5)
                    dcp = sm.tile([1, 512], F32, tag="dcp", name="dcp")
                    nc.vector.tensor_copy(dcp[:], pv_t[64:65, :])
                    rdf = sm.tile([1, 512], F32, tag="rdf", name="rdf")
                    nc.vector.reciprocal_approx_fast(rdf[:], dcp[:])
                    rd = sm.tile([1, 512], F32R, tag="rd", name="rd")
                    nc.vector.tensor_copy(rd[:], rdf[:])
                    bc = psX.tile([64, 512], F32, tag="x", name="bc",
                                  padded_shape=[P, 512])
                    nc.tensor.matmul(bc[:], onesr[:], rd[:], start=True, stop=True)
                    me = sm.tile([64, 512], F32, tag="me", name="me")
                    nc.vector.tensor_copy(me[:], pv_t[0:64, :])
                    yh = sm.tile([64, 512], BF16, tag="yh", name="yh")
                    if fast_p1:
                        y1 = sm.tile([64, 512], F32, tag="y1", name="y1")
                        nc.vector.scalar_tensor_tensor(
                            y1[:], me[:], eph[:, h:h + 1], bc[:],
                            ALU.mult, ALU.mult)
                        nc.vector.tensor_scalar_add(yh[:], y1[:],
                                                    ech[:, h:h + 1])
                    else:
                        nc.vector.tensor_tensor(me[:], me[:], bc[:], ALU.mult)
                        nc.vector.tensor_scalar_add(me[:], me[:],
                                                    cmh[:, h:h + 1])
                        nc.scalar.activation(me[:], me[:], AF.Ln)
                        nc.scalar.activation(yh[:], me[:], AF.Exp,
                                             scale=iph[:, h:h + 1],
                                             bias=zmh[:, h:h + 1])
                        nc.vector.tensor_scalar_add(yh[:], yh[:], -SHIFT)
                    base, ch = 64 * (h % 2), h // 2
                    qi_ = post_chain_qi[0]
                    nc.gpsimd.dma_start(
                        yT[base:base + 64, ch, qi_ * 512:(qi_ + 1) * 512], yh[:])

                post_chain_qi = [0]

                for hp in range(2):
                    h0, h1 = 2 * hp, 2 * hp + 1
                    ch = hp
                    for qi in range(NQ):
                        npair = 2 * (qi + 1)       # kt pairs (kt = 2a, 2a+1)
                        qsl = slice(qi * 512, (qi + 1) * 512)
                        pv = [psV.tile([65, 512], F32, tag="pv", name=f"pv{_i}")
                              for _i in range(2)]
                        prev = None
                        for a in range(npair):
                            ptile = []
                            # scores: 2 heads row-packed on the PE array
                            s_ps = [psS.tile([P, 1024], F32, tag="s",
                                             name=f"s{_i}") for _i in range(2)]
                            for half in range(2):
                                kt = 2 * a + half
                                ksl = slice(kt * P, (kt + 1) * P)
                                for i, h in enumerate((h0, h1)):
                                    nc.tensor.matmul(
                                        s_ps[i][:, half * 512:(half + 1) * 512],
                                        kTp[:, h, ksl],
                                        qTp[:, h, qsl],
                                        start=True, stop=True,
                                        skip_group_check=True,
                                    )
                            for i in range(2):
                                pt = att.tile([P, 1024], BF16, tag="pT",
                                              name="pt")
                                nc.scalar.activation(pt[:], s_ps[i][:], AF.Exp,
                                                     scale=SM_SCALE)
                                j0 = 2 * a - 4 * qi
                                if j0 >= 0:   # diagonal band: mask pair
                                    nc.vector.tensor_mul(
                                        pt[:],
                                        pt[:],
                                        cst[:, j0 * 512:(j0 + 2) * 512])
                                ptile.append(pt)
                            if prev is not None:
                                pa_, pp0, pp1 = prev
                                for i, ppt in enumerate((pp0, pp1)):
                                    for half in range(2):
                                        kt = 2 * pa_ + half
                                        nc.tensor.matmul(
                                            pv[i][:],
                                            vnat[:, (h0, h1)[i], kt, :],
                                            ppt[:, half * 512:(half + 1) * 512],
                                            start=(kt == 0), stop=False,
                                            skip_group_check=True,
                                        )
                            if a == 1 and pending:
                                for fn in pending:
                                    fn()
                                pending.clear()
                                if hp == 1 and qi > 0:
                                    for tq in range(4 * (qi - 1), 4 * qi):
                                        proj_group(tq)
                            prev = (a, ptile[0], ptile[1])
                        pa_, pp0, pp1 = prev
                        for i, ppt in enumerate((pp0, pp1)):
                            for half in range(2):
                                kt = 2 * pa_ + half
                                nc.tensor.matmul(
                                    pv[i][:],
                                    vnat[:, (h0, h1)[i], kt, :],
                                    ppt[:, half * 512:(half + 1) * 512],
                                    start=(kt == 0), stop=(half == 1),
                                    skip_group_check=True,
                                )

                        def mk(pv_t, h, qi):
                            def fn():
                                post_chain_qi[0] = qi
                                post_chain(pv_t, h)
                            return fn
                        pending.append(mk(pv[0], h0, qi))
                        pending.append(mk(pv[1], h1, qi))
                for fn in pending:
                    fn()
                pending.clear()
                for tq in range(12, 16):
                    proj_group(tq)


    nc.finalize()
    return nc


def _host_inputs(x, w_attn, b_attn, w_proj, p_param):
    """Build the 8 per-core input dicts."""
    bf16 = ml_dtypes.bfloat16
    ident = np.concatenate([np.eye(64, dtype=np.float32)] * 2, axis=0)
    xx = np.arange(P, dtype=np.int64)[:, None]
    yy = np.arange(512, dtype=np.int64)[None, :]
    masks = np.concatenate(
        [(yy - xx - P * j >= 0).astype(np.float32) for j in range(4)], axis=1)
    onesc = np.ones((P, NK), dtype=np.float32)
    cst = np.concatenate([masks, ident, onesc], axis=1).astype(bf16)
    onesr = np.ones((1, 64), dtype=np.float32)

    xts = [np.ascontiguousarray(x[b].T).astype(bf16) for b in range(B)]
    in_maps = []
    for core in range(8):
        b, hg = divmod(core, 4)
        cs = slice(hg * CL, (hg + 1) * CL)
        csC = slice(C + hg * CL, C + (hg + 1) * CL)
        cs2C = slice(2 * C + hg * CL, 2 * C + (hg + 1) * CL)
        in_maps.append({
            "xt": xts[b],
            "wq": np.ascontiguousarray(w_attn[:, cs]).astype(bf16),
            "wk": np.ascontiguousarray(w_attn[:, csC]).astype(bf16),
            "wv": np.ascontiguousarray(w_attn[:, cs2C]).astype(bf16),
            "wp": np.ascontiguousarray(w_proj[cs, :]).astype(bf16),
            "bps": np.ascontiguousarray(np.concatenate([
                b_attn[cs].reshape(2, P).T,
                b_attn[csC].reshape(2, P).T,
                (b_attn[cs2C] + SHIFT).reshape(2, P).T,
                p_param[cs].reshape(2, P).T,
            ], axis=1).astype(np.float32)),
            "cst": cst,
            "onesr": onesr,
        })
    return in_maps


def kernel(x, w_attn, b_attn, w_proj, b_proj, p_param, _trace=False):
    x = np.asarray(x, dtype=np.float32)
    w_attn = np.asarray(w_attn, dtype=np.float32)
    b_attn = np.asarray(b_attn, dtype=np.float32)
    w_proj = np.asarray(w_proj, dtype=np.float32)
    b_proj = np.asarray(b_proj, dtype=np.float32)
    p_param = np.asarray(p_param, dtype=np.float32)

    # p == 1 admits a cheaper final transform (no per-tile ln/exp)
    p_eff = np.sign(np.sign(p_param) + 0.5) * np.clip(np.abs(p_param),
                                                      P_MIN, P_MAX)
    fast_p1 = bool(np.all(p_eff == 1.0))

    key = ("nc", fast_p1)
    if key not in _CACHE:
        _CACHE[key] = _build(fast_p1)
    nc = _CACHE[key]

    in_maps = _host_inputs(x, w_attn, b_attn, w_proj, p_param)
    res = run_bass_kernel_spmd(nc, in_maps, core_ids=list(range(8)),
                               trace=_trace)
    _CACHE["last_result"] = res

    out = np.zeros((B, T, C), dtype=np.float32)
    for core in range(8):
        b = core // 4
        out[b] += res.results[core]["out_p"]
    out += b_proj[None, None, :]
    return out


if __name__ == "__main__":
    rng = np.random.default_rng(0)
    ins = {
        "x": rng.standard_normal((B, T, C), dtype=np.float32),
        "w_attn": (rng.standard_normal((C, 3 * C), dtype=np.float32) * 0.02),
        "b_attn": np.zeros(3 * C, np.float32),
        "w_proj": (rng.standard_normal((C, C), dtype=np.float32) * 0.02),
        "b_proj": np.zeros(C, np.float32),
        "p_param": np.ones(C, np.float32),
    }
    out = kernel(**ins)
    print("ran, out shape", out.shape, "finite:", np.isfinite(out).all())


# revision 21
# speedup vs baseline: 1.2158x; 1.0286x over previous
"""Trainium2 Bass kernel for nn_CausalGemAttention.

Reference computation (B=2, T=2048, C=1024, H=16, d=64):
    qkv = x @ w_attn + b_attn ; q,k,v = split(qkv)
    p = sign(sign(p_param)+0.5) * clamp(|p_param|, 1e-4, 1e3)
    vc = clip(|v + 5|, 1e-10); z = p*ln(vc); zmax = max_T(z); v' = exp(z - zmax)
    att = causal_softmax(q k^T / sqrt(d)); mean = att @ v'
    y = exp((zmax + ln(mean)) / p) - 5 ; out = y @ w_proj + b_proj

Sharding: 8 cores = 2 (batch) x 4 (head groups of 4 heads / 256 channels).
Each core computes qkv for its head group (contraction over full C), local
attention, and a partial projection (w_proj rows of its channels); host sums
the 4 partials per batch and adds b_proj.

Matmul operands are bf16 with fp32 PSUM accumulation.  To keep bf16 rounding
out of the softmax average (the +5 shift amplifies relative error of the
mean ~8x), v' is centered per channel before the PV matmul:
    mean = num''/den + cmid   with   v'' = v' - cmid
cmid is carried in fp32 and re-added exactly.  When p == 1 (the shipped
configuration) the final transform reduces to y = exp(zmax)*mean - 5 and is
computed without any per-tile ln/exp; a general-p fallback path exists.
"""

import sys
sys.path.insert(0, "/opt/trn_rl_repo")

import numpy as np
import ml_dtypes

import concourse.bacc as bacc
import concourse.tile as tile
from concourse import mybir
from concourse.bass_utils import run_bass_kernel_spmd

F32 = mybir.dt.float32
F32R = mybir.dt.float32r
BF16 = mybir.dt.bfloat16
AF = mybir.ActivationFunctionType
ALU = mybir.AluOpType
AX = mybir.AxisListType

B, T, C, H, D = 2, 2048, 1024, 16, 64
P = 128
CL = 256            # channels per core (4 heads x 64)
KC = C // P         # 8 contraction chunks for qkv
NQ = T // 512       # 4 query blocks of 512
NK = T // P         # 16 key tiles of 128
SHIFT = 5.0
P_MIN, P_MAX, V_MIN = 1e-4, 1e3, 1e-10
SM_SCALE = 1.0 / 8.0  # 1/sqrt(64)

_CACHE = {}


def _build(fast_p1):
    nc = bacc.Bacc("TRN2", target_bir_lowering=False, debug=False)

    xt_d = nc.dram_tensor("xt", [C, T], BF16, kind="ExternalInput")
    wq_d = nc.dram_tensor("wq", [C, CL], BF16, kind="ExternalInput")
    wk_d = nc.dram_tensor("wk", [C, CL], BF16, kind="ExternalInput")
    wv_d = nc.dram_tensor("wv", [C, CL], BF16, kind="ExternalInput")
    wp_d = nc.dram_tensor("wp", [CL, C], BF16, kind="ExternalInput")
    # bps = [bq(2) | bk(2) | bv5(2) | pp(2)] per chunk, fp32
    bps_d = nc.dram_tensor("bps", [P, 8], F32, kind="ExternalInput")
    # cst = [masks(4x512) | ident(64) | onesc(16)] packed along free dim
    cst_d = nc.dram_tensor("cst", [P, 2128], BF16, kind="ExternalInput")
    or_d = nc.dram_tensor("onesr", [1, 64], F32R, kind="ExternalInput")
    out_d = nc.dram_tensor("out_p", [T, C], F32, kind="ExternalOutput")

    with tile.TileContext(nc) as tc:
        with (
            tc.tile_pool(name="consts", bufs=1) as cp,
            tc.tile_pool(name="qk", bufs=1) as qkp,
            tc.tile_pool(name="vy", bufs=1) as vyp,
        ):
            cst = cp.tile([P, 2128], BF16)
            onesr = cp.tile([1, 64], F32R)
            bps = cp.tile([P, 8], F32)
            nc.sync.dma_start(bps[:], bps_d[:])
            ident = cst[:, 2048:2112]
            bq_sb = bps[:, 0:2]
            bk_sb = bps[:, 2:4]
            bv5_sb = bps[:, 4:6]
            pp_sb = bps[:, 6:8]

            # p = sign(sign(pp)+0.5) * clamp(|pp|, P_MIN, P_MAX); ip = 1/p
            sgn = cp.tile([P, 2], F32)
            ab = cp.tile([P, 2], F32)
            p_sb = cp.tile([P, 2], F32)
            # allcp packs [ip | zmaxp | cmid | ezp | ecp5] x 2 chunks
            allcp = cp.tile([P, 5, 2], F32)
            ip_sb = allcp[:, 0, :]
            zmaxp = allcp[:, 1, :]
            cmid = allcp[:, 2, :]
            ezp = allcp[:, 3, :]
            ecp5 = allcp[:, 4, :]
            nc.scalar.activation(sgn[:], pp_sb[:], AF.Sign)
            nc.vector.tensor_scalar_add(sgn[:], sgn[:], 0.5)
            nc.scalar.activation(sgn[:], sgn[:], AF.Sign)
            nc.scalar.activation(ab[:], pp_sb[:], AF.Abs)
            nc.vector.tensor_scalar(ab[:], ab[:], float(P_MIN), float(P_MAX),
                                    ALU.max, ALU.min)
            nc.vector.tensor_tensor(p_sb[:], sgn[:], ab[:], ALU.mult)
            nc.vector.reciprocal(ip_sb[:], p_sb[:])

            negzmax = cp.tile([P, 2], F32)
            zmin_sb = cp.tile([P, 2], F32)
            # allh: per-head [64,1] base-0 views of allcp, [64, const, head]
            allh = cp.tile([64, 5, 4], F32)
            iph = allh[:, 0, :]
            zmh = allh[:, 1, :]
            cmh = allh[:, 2, :]
            eph = allh[:, 3, :]
            ech = allh[:, 4, :]

            qT = qkp.tile([P, 2, T], BF16)   # q^T: [c%128, c//128, t]
            kT = qkp.tile([P, 2, T], BF16)
            vnat = vyp.tile([P, 4, NK, 65], BF16)  # [tk%128, head, tk//128, d|1]
            yT = vyp.tile([P, 2, T], BF16)
            wp_sb = vyp.tile([P, 2, C], BF16)
            # padded per-head q/k (K=128 with zero rows 64:127): K=64 matmuls
            # measure ~1.5x slower than K=128 on the PE, so pad instead.
            qTp = vyp.tile([P, 4, T], BF16)
            kTp = vyp.tile([P, 4, T], BF16)

            # ---------------- Phase A: qkv;  B: v transform + transposes -----
            with (
                tc.tile_pool(name="pA", bufs=1) as pA,
                tc.tile_pool(name="pB", bufs=1) as pB,
                tc.tile_pool(name="psA", bufs=6, space="PSUM") as psA,
            ):
                xt_sb = pA.tile([P, KC, T], BF16)
                wq_sb = pA.tile([P, KC, CL], BF16)
                wk_sb = pA.tile([P, KC, CL], BF16)
                wv_sb = pA.tile([P, KC, CL], BF16)
                nc.sync.dma_start(wv_sb[:], wv_d[:].rearrange("(a p) m -> p a m", p=P))
                nc.sync.dma_start(wq_sb[:], wq_d[:].rearrange("(a p) m -> p a m", p=P))
                nc.sync.dma_start(wk_sb[:], wk_d[:].rearrange("(a p) m -> p a m", p=P))
                for kc in range(KC):
                    for hf in range(2):
                        nc.sync.dma_start(
                            xt_sb[:, kc, hf * 1024:(hf + 1) * 1024],
                            xt_d[kc * P:(kc + 1) * P, hf * 1024:(hf + 1) * 1024])
                nc.sync.dma_start(cst[:], cst_d[:])
                nc.sync.dma_start(onesr[:], or_d[:])
                nc.sync.dma_start(wp_sb[:],
                                  wp_d[:].rearrange("(c p) n -> p c n", p=P))
                nc.vector.memset(qTp[64:128, :, :], 0.0)
                nc.vector.memset(kTp[64:128, :, :], 0.0)

                vT = pB.tile([P, 2, T], F32)
                vpT = pB.tile([P, 2, T], BF16)

                def qkv_group(wsb, kind, m, nt):
                    ps = psA.tile([P, 512], F32, tag="ev", name="ev")
                    for kc in range(KC):
                        nc.tensor.matmul(
                            ps[:],
                            wsb[:, kc, m * P:(m + 1) * P],
                            xt_sb[:, kc, nt * 512:(nt + 1) * 512],
                            start=(kc == 0), stop=(kc == KC - 1),
                        )
                    tsl = slice(nt * 512, (nt + 1) * 512)
                    if kind == "q":
                        nc.vector.tensor_scalar_add(
                            qT[:, m, tsl], ps[:], bq_sb[:, m:m + 1])
                    elif kind == "k":
                        nc.vector.tensor_scalar_add(
                            kT[:, m, tsl], ps[:], bk_sb[:, m:m + 1])
                    else:
                        # |v + b + SHIFT| directly out of PSUM
                        nc.scalar.activation(
                            vT[:, m, tsl], ps[:], AF.Abs,
                            bias=bv5_sb[:, m:m + 1])

                # v first so its transform overlaps the q/k matmuls
                for m in range(2):
                    for nt in range(NQ):
                        qkv_group(wv_sb, "v", m, nt)

                # transform: z = p*ln(clip(vc)); zmax/zmin; v'' = e^(z-zmax)-cmid
                for m in range(2):
                    nc.vector.tensor_scalar_max(vT[:, m, :], vT[:, m, :],
                                                float(V_MIN))
                    nc.scalar.activation(vT[:, m, :], vT[:, m, :], AF.Ln)
                for m in range(2):
                    nc.vector.tensor_scalar_mul(vT[:, m, :], vT[:, m, :],
                                                p_sb[:, m:m + 1])
                    nc.vector.tensor_reduce(negzmax[:, m:m + 1], vT[:, m, :], AX.X,
                                            op=ALU.max, negate=True)
                    nc.vector.tensor_reduce(zmin_sb[:, m:m + 1], vT[:, m, :], AX.X,
                                            op=ALU.min)
                    nc.vector.scalar_tensor_tensor(
                        zmaxp[:, m:m + 1], negzmax[:, m:m + 1], -1.0,
                        ip_sb[:, m:m + 1], ALU.mult, ALU.mult)
                for m in range(2):
                    # cmid = 0.5*(1 + exp(zmin - zmax))
                    nc.scalar.activation(cmid[:, m:m + 1], zmin_sb[:, m:m + 1],
                                         AF.Exp, bias=negzmax[:, m:m + 1])
                    nc.vector.tensor_scalar(cmid[:, m:m + 1], cmid[:, m:m + 1],
                                            1.0, 0.5, ALU.add, ALU.mult)
                    # ezp = exp(zmax); ecp5 = ezp*cmid - 5
                    nc.scalar.activation(ezp[:, m:m + 1], negzmax[:, m:m + 1],
                                         AF.Exp, scale=-1.0)
                    nc.vector.scalar_tensor_tensor(
                        ecp5[:, m:m + 1], ezp[:, m:m + 1], 0.0,
                        cmid[:, m:m + 1], ALU.bypass, ALU.mult)
                    nc.vector.tensor_scalar_add(ecp5[:, m:m + 1], ecp5[:, m:m + 1],
                                                -SHIFT)
                    # v' (fp32, in place over z) then centered bf16 copy
                    nc.scalar.activation(vT[:, m, :], vT[:, m, :], AF.Exp,
                                         bias=negzmax[:, m:m + 1])
                    nc.vector.tensor_scalar_sub(vpT[:, m, :], vT[:, m, :],
                                                cmid[:, m:m + 1])

                # q/k matmuls (PE work that overlaps the v transform above)
                for m in range(2):
                    for nt in range(NQ):
                        qkv_group(wq_sb, "q", m, nt)
                        qkv_group(wk_sb, "k", m, nt)
                    for h in (2 * m, 2 * m + 1):
                        base = 64 * (h % 2)
                        nc.gpsimd.dma_start(qTp[0:64, h, :],
                                            qT[base:base + 64, m, :])
                        nc.gpsimd.dma_start(kTp[0:64, h, :],
                                            kT[base:base + 64, m, :])

                # per-head constants at partition base 0: heads (0,2) from
                # chunk rows 0:64, heads (1,3) from rows 64:128
                nc.sync.dma_start(allh[:, :, 0::2], allcp[0:64, :, :])
                nc.sync.dma_start(allh[:, :, 1::2], allcp[64:128, :, :])
                for h in range(4):
                    nc.sync.dma_start(vnat[:, h, :, 64], cst_d[:, 2112:2128])

                # transpose v''^T [d, tk] -> vnat [tk, d], 8 k-tiles per bank
                for h in range(4):
                    base, ch = 64 * (h % 2), h // 2
                    for half in range(2):
                        trp = psA.tile([P, 512], BF16, tag="tr", name="trp", bufs=2)
                        for j in range(8):
                            kt = half * 8 + j
                            nc.tensor.transpose(
                                trp[:, j * 64:(j + 1) * 64],
                                vpT[base:base + 64, ch, kt * P:(kt + 1) * P],
                                cst[base:base + 64, 2048:2112],
                            )
                        nc.vector.tensor_copy(
                            vnat[:, h, half * 8:(half + 1) * 8, 0:64],
                            trp[:].rearrange("p (a b) -> p a b", a=8),
                        )

            # ---------------- Phase C: attention ------------------------------
            with (
                tc.tile_pool(name="att", bufs=5) as att,
                tc.tile_pool(name="small", bufs=3) as sm,
                tc.tile_pool(name="outp", bufs=3) as op_,
                tc.tile_pool(name="psS", bufs=2, space="PSUM") as psS,
                tc.tile_pool(name="psV", bufs=2, space="PSUM") as psV,
                tc.tile_pool(name="psX", bufs=2, space="PSUM") as psX,
            ):
                pending = []   # deferred post-chains (emitted mid next q-block)

                def proj_group(tq):
                    po = op_.tile([P, C], F32, tag="po", name="po")
                    for nh in range(2):
                        pj = psX.tile([P, 512], F32, tag="x", name="pj")
                        for c in range(2):
                            nc.tensor.matmul(
                                pj[:],
                                yT[:, c, tq * P:(tq + 1) * P],
                                wp_sb[:, c, nh * 512:(nh + 1) * 512],
                                start=(c == 0), stop=(c == 1),
                            )
                        if nh == 0:
                            nc.scalar.activation(po[:, 0:512], pj[:], AF.Copy)
                        else:
                            nc.vector.tensor_copy(po[:, 512:1024], pj[:])
                    nc.gpsimd.dma_start(out_d[tq * P:(tq + 1) * P, :], po[:])

                def post_chain(pv_t, h):
                    # mean'' = num''/den ; y = ezp*mean'' + (ezp*cmid - 5)
                    dcp = sm.tile([1, 512], F32, tag="dcp", name="dcp")
                    nc.vector.tensor_copy(dcp[:], pv_t[64:65, :])
                    rdf = sm.tile([1, 512], F32, tag="rdf", name="rdf")
                    nc.vector.reciprocal_approx_fast(rdf[:], dcp[:])
                    rd = sm.tile([1, 512], F32R, tag="rd", name="rd")
                    nc.vector.tensor_copy(rd[:], rdf[:])
                    bc = psX.tile([64, 512], F32, tag="x", name="bc",
                                  padded_shape=[P, 512])
                    nc.tensor.matmul(bc[:], onesr[:], rd[:], start=True, stop=True)
                    me = sm.tile([64, 512], F32, tag="me", name="me")
                    nc.vector.tensor_copy(me[:], pv_t[0:64, :])
                    yh = sm.tile([64, 512], BF16, tag="yh", name="yh")
                    if fast_p1:
                        y1 = sm.tile([64, 512], F32, tag="y1", name="y1")
                        nc.vector.scalar_tensor_tensor(
                            y1[:], me[:], eph[:, h:h + 1], bc[:],
                            ALU.mult, ALU.mult)
                        nc.vector.tensor_scalar_add(yh[:], y1[:],
                                                    ech[:, h:h + 1])
                    else:
                        nc.vector.tensor_tensor(me[:], me[:], bc[:], ALU.mult)
                        nc.vector.tensor_scalar_add(me[:], me[:],
                                                    cmh[:, h:h + 1])
                        nc.scalar.activation(me[:], me[:], AF.Ln)
                        nc.scalar.activation(yh[:], me[:], AF.Exp,
                                             scale=iph[:, h:h + 1],
                                             bias=zmh[:, h:h + 1])
                        nc.vector.tensor_scalar_add(yh[:], yh[:], -SHIFT)
                    base, ch = 64 * (h % 2), h // 2
                    qi_ = post_chain_qi[0]
                    nc.gpsimd.dma_start(
                        yT[base:base + 64, ch, qi_ * 512:(qi_ + 1) * 512], yh[:])

                post_chain_qi = [0]

                for hp in range(2):
                    h0, h1 = 2 * hp, 2 * hp + 1
                    ch = hp
                    for qi in range(NQ):
                        npair = 2 * (qi + 1)       # kt pairs (kt = 2a, 2a+1)
                        qsl = slice(qi * 512, (qi + 1) * 512)
                        pv = [psV.tile([65, 512], F32, tag="pv", name=f"pv{_i}")
                              for _i in range(2)]
                        prev = None
                        for a in range(npair):
                            ptile = []
                            # scores: 2 heads row-packed on the PE array
                            s_ps = [psS.tile([P, 1024], F32, tag="s",
                                             name=f"s{_i}") for _i in range(2)]
                            j0 = 2 * a - 4 * qi
                            # causal column offset: tile kt only has valid
                            # scores for tq >= 128*j (j = kt - 4*qi)
                            off0 = P * max(j0, 0)
                            for half in range(2):
                                kt = 2 * a + half
                                off = P * max(kt - 4 * qi, 0)
                                ksl = slice(kt * P, (kt + 1) * P)
                                qsub = slice(qi * 512 + off, (qi + 1) * 512)
                                for i, h in enumerate((h0, h1)):
                                    nc.tensor.matmul(
                                        s_ps[i][:, half * 512 + off:
                                                (half + 1) * 512],
                                        kTp[:, h, ksl],
                                        qTp[:, h, qsub],
                                        start=True, stop=True,
                                        skip_group_check=True,
                                    )
                            for i in range(2):
                                pt = att.tile([P, 1024], BF16, tag="pT",
                                              name="pt")
                                nc.scalar.activation(pt[:, off0:1024],
                                                     s_ps[i][:, off0:1024],
                                                     AF.Exp, scale=SM_SCALE)
                                if j0 >= 0:   # diagonal band: mask pair
                                    nc.vector.tensor_mul(
                                        pt[:, off0:1024],
                                        pt[:, off0:1024],
                                        cst[:, j0 * 512 + off0:
                                            (j0 + 2) * 512])
                                ptile.append(pt)
                            if prev is not None:
                                pa_, pp0, pp1 = prev
                                for i, ppt in enumerate((pp0, pp1)):
                                    for half in range(2):
                                        kt = 2 * pa_ + half
                                        o_ = P * max(kt - 4 * qi, 0)
                                        nc.tensor.matmul(
                                            pv[i][:, o_:512],
                                            vnat[:, (h0, h1)[i], kt, :],
                                            ppt[:, half * 512 + o_:
                                                (half + 1) * 512],
                                            start=(kt == 0), stop=False,
                                            skip_group_check=True,
                                        )
                            if a == 1 and pending:
                                for fn in pending:
                                    fn()
                                pending.clear()
                                if hp == 1 and qi > 0:
                                    for tq in range(4 * (qi - 1), 4 * qi):
                                        proj_group(tq)
                            prev = (a, ptile[0], ptile[1])
                        pa_, pp0, pp1 = prev
                        for i, ppt in enumerate((pp0, pp1)):
                            for half in range(2):
                                kt = 2 * pa_ + half
                                o_ = P * max(kt - 4 * qi, 0)
                                nc.tensor.matmul(
                                    pv[i][:, o_:512],
                                    vnat[:, (h0, h1)[i], kt, :],
                                    ppt[:, half * 512 + o_:(half + 1) * 512],
                                    start=(kt == 0), stop=(half == 1),
                                    skip_group_check=True,
                                )

                        def mk(pv_t, h, qi):
                            def fn():
                                post_chain_qi[0] = qi
                                post_chain(pv_t, h)
                            return fn
                        pending.append(mk(pv[0], h0, qi))
                        pending.append(mk(pv[1], h1, qi))
                for fn in pending:
                    fn()
                pending.clear()
                for tq in range(12, 16):
                    proj_group(tq)


    nc.finalize()
    return nc


def _host_inputs(x, w_attn, b_attn, w_proj, p_param):
    """Build the 8 per-core input dicts."""
    bf16 = ml_dtypes.bfloat16
    ident = np.concatenate([np.eye(64, dtype=np.float32)] * 2, axis=0)
    xx = np.arange(P, dtype=np.int64)[:, None]
    yy = np.arange(512, dtype=np.int64)[None, :]
    masks = np.concatenate(
        [(yy - xx - P * j >= 0).astype(np.float32) for j in range(4)], axis=1)
    onesc = np.ones((P, NK), dtype=np.float32)
    cst = np.concatenate([masks, ident, onesc], axis=1).astype(bf16)
    onesr = np.ones((1, 64), dtype=np.float32)

    xts = [np.ascontiguousarray(x[b].T).astype(bf16) for b in range(B)]
    in_maps = []
    for core in range(8):
        b, hg = divmod(core, 4)
        cs = slice(hg * CL, (hg + 1) * CL)
        csC = slice(C + hg * CL, C + (hg + 1) * CL)
        cs2C = slice(2 * C + hg * CL, 2 * C + (hg + 1) * CL)
        in_maps.append({
            "xt": xts[b],
            "wq": np.ascontiguousarray(w_attn[:, cs]).astype(bf16),
            "wk": np.ascontiguousarray(w_attn[:, csC]).astype(bf16),
            "wv": np.ascontiguousarray(w_attn[:, cs2C]).astype(bf16),
            "wp": np.ascontiguousarray(w_proj[cs, :]).astype(bf16),
            "bps": np.ascontiguousarray(np.concatenate([
                b_attn[cs].reshape(2, P).T,
                b_attn[csC].reshape(2, P).T,
                (b_attn[cs2C] + SHIFT).reshape(2, P).T,
                p_param[cs].reshape(2, P).T,
            ], axis=1).astype(np.float32)),
            "cst": cst,
            "onesr": onesr,
        })
    return in_maps


def kernel(x, w_attn, b_attn, w_proj, b_proj, p_param, _trace=False):
    x = np.asarray(x, dtype=np.float32)
    w_attn = np.asarray(w_attn, dtype=np.float32)
    b_attn = np.asarray(b_attn, dtype=np.float32)
    w_proj = np.asarray(w_proj, dtype=np.float32)
    b_proj = np.asarray(b_proj, dtype=np.float32)
    p_param = np.asarray(p_param, dtype=np.float32)

    # p == 1 admits a cheaper final transform (no per-tile ln/exp)
    p_eff = np.sign(np.sign(p_param) + 0.5) * np.clip(np.abs(p_param),
                                                      P_MIN, P_MAX)
    fast_p1 = bool(np.all(p_eff == 1.0))

    key = ("nc", fast_p1)
    if key not in _CACHE:
        _CACHE[key] = _build(fast_p1)
    nc = _CACHE[key]

    in_maps = _host_inputs(x, w_attn, b_attn, w_proj, p_param)
    res = run_bass_kernel_spmd(nc, in_maps, core_ids=list(range(8)),
                               trace=_trace)
    _CACHE["last_result"] = res

    out = np.zeros((B, T, C), dtype=np.float32)
    for core in range(8):
        b = core // 4
        out[b] += res.results[core]["out_p"]
    out += b_proj[None, None, :]
    return out


if __name__ == "__main__":
    rng = np.random.default_rng(0)
    ins = {
        "x": rng.standard_normal((B, T, C), dtype=np.float32),
        "w_attn": (rng.standard_normal((C, 3 * C), dtype=np.float32) * 0.02),
        "b_attn": np.zeros(3 * C, np.float32),
        "w_proj": (rng.standard_normal((C, C), dtype=np.float32) * 0.02),
        "b_proj": np.zeros(C, np.float32),
        "p_param": np.ones(C, np.float32),
    }
    out = kernel(**ins)
    print("ran, out shape", out.shape, "finite:", np.isfinite(out).all())


# revision 22
# speedup vs baseline: 1.2186x; 1.0023x over previous
"""Trainium2 Bass kernel for nn_CausalGemAttention.

Reference computation (B=2, T=2048, C=1024, H=16, d=64):
    qkv = x @ w_attn + b_attn ; q,k,v = split(qkv)
    p = sign(sign(p_param)+0.5) * clamp(|p_param|, 1e-4, 1e3)
    vc = clip(|v + 5|, 1e-10); z = p*ln(vc); zmax = max_T(z); v' = exp(z - zmax)
    att = causal_softmax(q k^T / sqrt(d)); mean = att @ v'
    y = exp((zmax + ln(mean)) / p) - 5 ; out = y @ w_proj + b_proj

Sharding: 8 cores = 2 (batch) x 4 (head groups of 4 heads / 256 channels).
Each core computes qkv for its head group (contraction over full C), local
attention, and a partial projection (w_proj rows of its channels); host sums
the 4 partials per batch and adds b_proj.

Matmul operands are bf16 with fp32 PSUM accumulation.  To keep bf16 rounding
out of the softmax average (the +5 shift amplifies relative error of the
mean ~8x), v' is centered per channel before the PV matmul:
    mean = num''/den + cmid   with   v'' = v' - cmid
cmid is carried in fp32 and re-added exactly.  When p == 1 (the shipped
configuration) the final transform reduces to y = exp(zmax)*mean - 5 and is
computed without any per-tile ln/exp; a general-p fallback path exists.

Performance notes (per core, ~197us on HW):
  - flash-attention-free layout: scores computed as S^T = K^T.T Q^T with
    keys on PSUM partitions, so softmax denominators come from a ones
    column appended to v'' in the PV matmul (out = [v''|1].T @ P).
  - q/k are zero-padded to K=128 contraction (K=64 matmuls measure ~1.5x
    slower per N-cycle on the PE).
  - softmax exp runs on ScalarE over [128,1024] PSUM pairs to amortize the
    ~352-cycle ACT op overhead; Ln/Exp stay in one ACT table set.
  - the 1/den reciprocal uses reciprocal_approx_fast + a K=1 outer-product
    broadcast on the PE; post-chains are emitted one q-block late so the PE
    never stalls on the DVE chain.
  - causal sub-ranges: diagonal-band tiles only compute/exp/accumulate the
    valid column range.
  - projection is emitted interleaved with the tail of attention.
"""

import sys
sys.path.insert(0, "/opt/trn_rl_repo")

import numpy as np
import ml_dtypes

import concourse.bacc as bacc
import concourse.tile as tile
from concourse import mybir
from concourse.bass_utils import run_bass_kernel_spmd

F32 = mybir.dt.float32
F32R = mybir.dt.float32r
BF16 = mybir.dt.bfloat16
AF = mybir.ActivationFunctionType
ALU = mybir.AluOpType
AX = mybir.AxisListType

B, T, C, H, D = 2, 2048, 1024, 16, 64
P = 128
CL = 256            # channels per core (4 heads x 64)
KC = C // P         # 8 contraction chunks for qkv
NQ = T // 512       # 4 query blocks of 512
NK = T // P         # 16 key tiles of 128
SHIFT = 5.0
P_MIN, P_MAX, V_MIN = 1e-4, 1e3, 1e-10
SM_SCALE = 1.0 / 8.0  # 1/sqrt(64)

_CACHE = {}


def _build(fast_p1):
    nc = bacc.Bacc("TRN2", target_bir_lowering=False, debug=False)

    xt_d = nc.dram_tensor("xt", [C, T], BF16, kind="ExternalInput")
    wq_d = nc.dram_tensor("wq", [C, CL], BF16, kind="ExternalInput")
    wk_d = nc.dram_tensor("wk", [C, CL], BF16, kind="ExternalInput")
    wv_d = nc.dram_tensor("wv", [C, CL], BF16, kind="ExternalInput")
    wp_d = nc.dram_tensor("wp", [CL, C], BF16, kind="ExternalInput")
    # bps = [bq(2) | bk(2) | bv5(2) | pp(2)] per chunk, fp32
    bps_d = nc.dram_tensor("bps", [P, 8], F32, kind="ExternalInput")
    # cst = [masks(4x512) | ident(64) | onesc(16)] packed along free dim
    cst_d = nc.dram_tensor("cst", [P, 2128], BF16, kind="ExternalInput")
    or_d = nc.dram_tensor("onesr", [1, 64], F32R, kind="ExternalInput")
    out_d = nc.dram_tensor("out_p", [T, C], F32, kind="ExternalOutput")

    with tile.TileContext(nc) as tc:
        with (
            tc.tile_pool(name="consts", bufs=1) as cp,
            tc.tile_pool(name="qk", bufs=1) as qkp,
            tc.tile_pool(name="vy", bufs=1) as vyp,
        ):
            cst = cp.tile([P, 2128], BF16)
            onesr = cp.tile([1, 64], F32R)
            bps = cp.tile([P, 8], F32)
            nc.sync.dma_start(bps[:], bps_d[:])
            ident = cst[:, 2048:2112]
            bq_sb = bps[:, 0:2]
            bk_sb = bps[:, 2:4]
            bv5_sb = bps[:, 4:6]
            pp_sb = bps[:, 6:8]

            # p = sign(sign(pp)+0.5) * clamp(|pp|, P_MIN, P_MAX); ip = 1/p
            sgn = cp.tile([P, 2], F32)
            ab = cp.tile([P, 2], F32)
            p_sb = cp.tile([P, 2], F32)
            # allcp packs [ip | zmaxp | cmid | ezp | ecp5] x 2 chunks
            allcp = cp.tile([P, 5, 2], F32)
            ip_sb = allcp[:, 0, :]
            zmaxp = allcp[:, 1, :]
            cmid = allcp[:, 2, :]
            ezp = allcp[:, 3, :]
            ecp5 = allcp[:, 4, :]
            nc.scalar.activation(sgn[:], pp_sb[:], AF.Sign)
            nc.vector.tensor_scalar_add(sgn[:], sgn[:], 0.5)
            nc.scalar.activation(sgn[:], sgn[:], AF.Sign)
            nc.scalar.activation(ab[:], pp_sb[:], AF.Abs)
            nc.vector.tensor_scalar(ab[:], ab[:], float(P_MIN), float(P_MAX),
                                    ALU.max, ALU.min)
            nc.vector.tensor_tensor(p_sb[:], sgn[:], ab[:], ALU.mult)
            nc.vector.reciprocal(ip_sb[:], p_sb[:])

            negzmax = cp.tile([P, 2], F32)
            zmin_sb = cp.tile([P, 2], F32)
            # allh: per-head [64,1] base-0 views of allcp, [64, const, head]
            allh = cp.tile([64, 5, 4], F32)
            iph = allh[:, 0, :]
            zmh = allh[:, 1, :]
            cmh = allh[:, 2, :]
            eph = allh[:, 3, :]
            ech = allh[:, 4, :]

            qT = qkp.tile([P, 2, T], BF16)   # q^T: [c%128, c//128, t]
            kT = qkp.tile([P, 2, T], BF16)
            vnat = vyp.tile([P, 4, NK, 65], BF16)  # [tk%128, head, tk//128, d|1]
            yT = vyp.tile([P, 2, T], BF16)
            wp_sb = vyp.tile([P, 2, C], BF16)
            # padded per-head q/k (K=128 with zero rows 64:127): K=64 matmuls
            # measure ~1.5x slower than K=128 on the PE, so pad instead.
            qTp = vyp.tile([P, 4, T], BF16)
            kTp = vyp.tile([P, 4, T], BF16)

            # ---------------- Phase A: qkv;  B: v transform + transposes -----
            with (
                tc.tile_pool(name="pA", bufs=1) as pA,
                tc.tile_pool(name="pB", bufs=1) as pB,
                tc.tile_pool(name="psA", bufs=6, space="PSUM") as psA,
            ):
                xt_sb = pA.tile([P, KC, T], BF16)
                wq_sb = pA.tile([P, KC, CL], BF16)
                wk_sb = pA.tile([P, KC, CL], BF16)
                wv_sb = pA.tile([P, KC, CL], BF16)
                nc.sync.dma_start(wv_sb[:], wv_d[:].rearrange("(a p) m -> p a m", p=P))
                nc.sync.dma_start(wq_sb[:], wq_d[:].rearrange("(a p) m -> p a m", p=P))
                nc.sync.dma_start(wk_sb[:], wk_d[:].rearrange("(a p) m -> p a m", p=P))
                for kc in range(KC):
                    for hf in range(2):
                        nc.sync.dma_start(
                            xt_sb[:, kc, hf * 1024:(hf + 1) * 1024],
                            xt_d[kc * P:(kc + 1) * P, hf * 1024:(hf + 1) * 1024])
                nc.sync.dma_start(cst[:], cst_d[:])
                nc.sync.dma_start(onesr[:], or_d[:])
                nc.sync.dma_start(wp_sb[:],
                                  wp_d[:].rearrange("(c p) n -> p c n", p=P))
                nc.vector.memset(qTp[64:128, :, :], 0.0)
                nc.vector.memset(kTp[64:128, :, :], 0.0)

                vT = pB.tile([P, 2, T], F32)
                vpT = pB.tile([P, 2, T], BF16)

                def qkv_group(wsb, kind, m, nt):
                    ps = psA.tile([P, 512], F32, tag="ev", name="ev")
                    for kc in range(KC):
                        nc.tensor.matmul(
                            ps[:],
                            wsb[:, kc, m * P:(m + 1) * P],
                            xt_sb[:, kc, nt * 512:(nt + 1) * 512],
                            start=(kc == 0), stop=(kc == KC - 1),
                        )
                    tsl = slice(nt * 512, (nt + 1) * 512)
                    if kind == "q":
                        nc.vector.tensor_scalar_add(
                            qT[:, m, tsl], ps[:], bq_sb[:, m:m + 1])
                    elif kind == "k":
                        nc.vector.tensor_scalar_add(
                            kT[:, m, tsl], ps[:], bk_sb[:, m:m + 1])
                    else:
                        # |v + b + SHIFT| directly out of PSUM
                        nc.scalar.activation(
                            vT[:, m, tsl], ps[:], AF.Abs,
                            bias=bv5_sb[:, m:m + 1])

                # v first so its transform overlaps the q/k matmuls
                for m in range(2):
                    for nt in range(NQ):
                        qkv_group(wv_sb, "v", m, nt)

                # transform: z = p*ln(clip(vc)); zmax/zmin; v'' = e^(z-zmax)-cmid
                for m in range(2):
                    nc.vector.tensor_scalar_max(vT[:, m, :], vT[:, m, :],
                                                float(V_MIN))
                    nc.scalar.activation(vT[:, m, :], vT[:, m, :], AF.Ln)
                for m in range(2):
                    nc.vector.tensor_scalar_mul(vT[:, m, :], vT[:, m, :],
                                                p_sb[:, m:m + 1])
                    nc.vector.tensor_reduce(negzmax[:, m:m + 1], vT[:, m, :], AX.X,
                                            op=ALU.max, negate=True)
                    nc.vector.tensor_reduce(zmin_sb[:, m:m + 1], vT[:, m, :], AX.X,
                                            op=ALU.min)
                    nc.vector.scalar_tensor_tensor(
                        zmaxp[:, m:m + 1], negzmax[:, m:m + 1], -1.0,
                        ip_sb[:, m:m + 1], ALU.mult, ALU.mult)
                for m in range(2):
                    # cmid = 0.5*(1 + exp(zmin - zmax))
                    nc.scalar.activation(cmid[:, m:m + 1], zmin_sb[:, m:m + 1],
                                         AF.Exp, bias=negzmax[:, m:m + 1])
                    nc.vector.tensor_scalar(cmid[:, m:m + 1], cmid[:, m:m + 1],
                                            1.0, 0.5, ALU.add, ALU.mult)
                    # ezp = exp(zmax); ecp5 = ezp*cmid - 5
                    nc.scalar.activation(ezp[:, m:m + 1], negzmax[:, m:m + 1],
                                         AF.Exp, scale=-1.0)
                    nc.vector.scalar_tensor_tensor(
                        ecp5[:, m:m + 1], ezp[:, m:m + 1], 0.0,
                        cmid[:, m:m + 1], ALU.bypass, ALU.mult)
                    nc.vector.tensor_scalar_add(ecp5[:, m:m + 1], ecp5[:, m:m + 1],
                                                -SHIFT)
                    # v' (fp32, in place over z) then centered bf16 copy
                    nc.scalar.activation(vT[:, m, :], vT[:, m, :], AF.Exp,
                                         bias=negzmax[:, m:m + 1])
                    nc.vector.tensor_scalar_sub(vpT[:, m, :], vT[:, m, :],
                                                cmid[:, m:m + 1])

                # q/k matmuls (PE work that overlaps the v transform above)
                for m in range(2):
                    for nt in range(NQ):
                        qkv_group(wq_sb, "q", m, nt)
                        qkv_group(wk_sb, "k", m, nt)
                    for h in (2 * m, 2 * m + 1):
                        base = 64 * (h % 2)
                        nc.gpsimd.dma_start(qTp[0:64, h, :],
                                            qT[base:base + 64, m, :])
                        nc.gpsimd.dma_start(kTp[0:64, h, :],
                                            kT[base:base + 64, m, :])

                # per-head constants at partition base 0: heads (0,2) from
                # chunk rows 0:64, heads (1,3) from rows 64:128
                nc.sync.dma_start(allh[:, :, 0::2], allcp[0:64, :, :])
                nc.sync.dma_start(allh[:, :, 1::2], allcp[64:128, :, :])
                for h in range(4):
                    nc.sync.dma_start(vnat[:, h, :, 64], cst_d[:, 2112:2128])

                # transpose v''^T [d, tk] -> vnat [tk, d], 8 k-tiles per bank
                for h in range(4):
                    base, ch = 64 * (h % 2), h // 2
                    for half in range(2):
                        trp = psA.tile([P, 512], BF16, tag="tr", name="trp", bufs=2)
                        for j in range(8):
                            kt = half * 8 + j
                            nc.tensor.transpose(
                                trp[:, j * 64:(j + 1) * 64],
                                vpT[base:base + 64, ch, kt * P:(kt + 1) * P],
                                cst[base:base + 64, 2048:2112],
                            )
                        nc.vector.tensor_copy(
                            vnat[:, h, half * 8:(half + 1) * 8, 0:64],
                            trp[:].rearrange("p (a b) -> p a b", a=8),
                        )

            # ---------------- Phase C: attention ------------------------------
            with (
                tc.tile_pool(name="att", bufs=5) as att,
                tc.tile_pool(name="small", bufs=3) as sm,
                tc.tile_pool(name="outp", bufs=3) as op_,
                tc.tile_pool(name="psS", bufs=2, space="PSUM") as psS,
                tc.tile_pool(name="psV", bufs=2, space="PSUM") as psV,
                tc.tile_pool(name="psX", bufs=2, space="PSUM") as psX,
            ):
                pending = []   # deferred post-chains (emitted mid next q-block)

                def proj_group(tq):
                    po = op_.tile([P, C], F32, tag="po", name="po")
                    for nh in range(2):
                        pj = psX.tile([P, 512], F32, tag="x", name="pj")
                        for c in range(2):
                            nc.tensor.matmul(
                                pj[:],
                                yT[:, c, tq * P:(tq + 1) * P],
                                wp_sb[:, c, nh * 512:(nh + 1) * 512],
                                start=(c == 0), stop=(c == 1),
                            )
                        if nh == 0:
                            nc.scalar.activation(po[:, 0:512], pj[:], AF.Copy)
                        else:
                            nc.vector.tensor_copy(po[:, 512:1024], pj[:])
                    nc.gpsimd.dma_start(out_d[tq * P:(tq + 1) * P, :], po[:])

                def post_chain(pv_t, h):
                    # mean'' = num''/den ; y = ezp*mean'' + (ezp*cmid - 5)
                    dcp = sm.tile([1, 512], F32, tag="dcp", name="dcp")
                    nc.vector.tensor_copy(dcp[:], pv_t[64:65, :])
                    rdf = sm.tile([1, 512], F32, tag="rdf", name="rdf")
                    nc.vector.reciprocal_approx_fast(rdf[:], dcp[:])
                    rd = sm.tile([1, 512], F32R, tag="rd", name="rd")
                    nc.vector.tensor_copy(rd[:], rdf[:])
                    bc = psX.tile([64, 512], F32, tag="x", name="bc",
                                  padded_shape=[P, 512])
                    nc.tensor.matmul(bc[:], onesr[:], rd[:], start=True, stop=True)
                    me = sm.tile([64, 512], F32, tag="me", name="me")
                    nc.vector.tensor_copy(me[:], pv_t[0:64, :])
                    yh = sm.tile([64, 512], BF16, tag="yh", name="yh")
                    if fast_p1:
                        y1 = sm.tile([64, 512], F32, tag="y1", name="y1")
                        nc.vector.scalar_tensor_tensor(
                            y1[:], me[:], eph[:, h:h + 1], bc[:],
                            ALU.mult, ALU.mult)
                        nc.vector.tensor_scalar_add(yh[:], y1[:],
                                                    ech[:, h:h + 1])
                    else:
                        nc.vector.tensor_tensor(me[:], me[:], bc[:], ALU.mult)
                        nc.vector.tensor_scalar_add(me[:], me[:],
                                                    cmh[:, h:h + 1])
                        nc.scalar.activation(me[:], me[:], AF.Ln)
                        nc.scalar.activation(yh[:], me[:], AF.Exp,
                                             scale=iph[:, h:h + 1],
                                             bias=zmh[:, h:h + 1])
                        nc.vector.tensor_scalar_add(yh[:], yh[:], -SHIFT)
                    base, ch = 64 * (h % 2), h // 2
                    qi_ = post_chain_qi[0]
                    nc.gpsimd.dma_start(
                        yT[base:base + 64, ch, qi_ * 512:(qi_ + 1) * 512], yh[:])

                post_chain_qi = [0]

                for hp in range(2):
                    h0, h1 = 2 * hp, 2 * hp + 1
                    ch = hp
                    for qi in range(NQ):
                        npair = 2 * (qi + 1)       # kt pairs (kt = 2a, 2a+1)
                        qsl = slice(qi * 512, (qi + 1) * 512)
                        pv = [psV.tile([65, 512], F32, tag="pv", name=f"pv{_i}")
                              for _i in range(2)]
                        prev = None
                        for a in range(npair):
                            ptile = []
                            s_ps = [psS.tile([P, 1024], F32, tag="s",
                                             name=f"s{_i}") for _i in range(2)]
                            j0 = 2 * a - 4 * qi
                            # causal column offset: tile kt only has valid
                            # scores for tq >= 128*j (j = kt - 4*qi)
                            off0 = P * max(j0, 0)
                            for half in range(2):
                                kt = 2 * a + half
                                off = P * max(kt - 4 * qi, 0)
                                ksl = slice(kt * P, (kt + 1) * P)
                                qsub = slice(qi * 512 + off, (qi + 1) * 512)
                                for i, h in enumerate((h0, h1)):
                                    nc.tensor.matmul(
                                        s_ps[i][:, half * 512 + off:
                                                (half + 1) * 512],
                                        kTp[:, h, ksl],
                                        qTp[:, h, qsub],
                                        start=True, stop=True,
                                        skip_group_check=True,
                                    )
                            for i in range(2):
                                pt = att.tile([P, 1024], BF16, tag="pT",
                                              name="pt")
                                nc.scalar.activation(pt[:, off0:1024],
                                                     s_ps[i][:, off0:1024],
                                                     AF.Exp, scale=SM_SCALE)
                                if j0 >= 0:   # diagonal band: mask pair
                                    nc.vector.tensor_mul(
                                        pt[:, off0:1024],
                                        pt[:, off0:1024],
                                        cst[:, j0 * 512 + off0:
                                            (j0 + 2) * 512])
                                ptile.append(pt)
                            if prev is not None:
                                pa_, pp0, pp1 = prev
                                for i, ppt in enumerate((pp0, pp1)):
                                    for half in range(2):
                                        kt = 2 * pa_ + half
                                        o_ = P * max(kt - 4 * qi, 0)
                                        nc.tensor.matmul(
                                            pv[i][:, o_:512],
                                            vnat[:, (h0, h1)[i], kt, :],
                                            ppt[:, half * 512 + o_:
                                                (half + 1) * 512],
                                            start=(kt == 0), stop=False,
                                            skip_group_check=True,
                                        )
                            if a == 1 and pending:
                                for fn in pending:
                                    fn()
                                pending.clear()
                                if hp == 1 and qi > 0:
                                    for tq in range(4 * (qi - 1), 4 * qi):
                                        proj_group(tq)
                            prev = (a, ptile[0], ptile[1])
                        pa_, pp0, pp1 = prev
                        for i, ppt in enumerate((pp0, pp1)):
                            for half in range(2):
                                kt = 2 * pa_ + half
                                o_ = P * max(kt - 4 * qi, 0)
                                nc.tensor.matmul(
                                    pv[i][:, o_:512],
                                    vnat[:, (h0, h1)[i], kt, :],
                                    ppt[:, half * 512 + o_:(half + 1) * 512],
                                    start=(kt == 0), stop=(half == 1),
                                    skip_group_check=True,
                                )

                        def mk(pv_t, h, qi):
                            def fn():
                                post_chain_qi[0] = qi
                                post_chain(pv_t, h)
                            return fn
                        pending.append(mk(pv[0], h0, qi))
                        pending.append(mk(pv[1], h1, qi))
                for fn in pending:
                    fn()
                pending.clear()
                for tq in range(12, 16):
                    proj_group(tq)


    nc.finalize()
    return nc


def _host_inputs(x, w_attn, b_attn, w_proj, p_param):
    """Build the 8 per-core input dicts."""
    bf16 = ml_dtypes.bfloat16
    ident = np.concatenate([np.eye(64, dtype=np.float32)] * 2, axis=0)
    xx = np.arange(P, dtype=np.int64)[:, None]
    yy = np.arange(512, dtype=np.int64)[None, :]
    masks = np.concatenate(
        [(yy - xx - P * j >= 0).astype(np.float32) for j in range(4)], axis=1)
    onesc = np.ones((P, NK), dtype=np.float32)
    cst = np.concatenate([masks, ident, onesc], axis=1).astype(bf16)
    onesr = np.ones((1, 64), dtype=np.float32)

    xts = [np.ascontiguousarray(x[b].T).astype(bf16) for b in range(B)]
    in_maps = []
    for core in range(8):
        b, hg = divmod(core, 4)
        cs = slice(hg * CL, (hg + 1) * CL)
        csC = slice(C + hg * CL, C + (hg + 1) * CL)
        cs2C = slice(2 * C + hg * CL, 2 * C + (hg + 1) * CL)
        in_maps.append({
            "xt": xts[b],
            "wq": np.ascontiguousarray(w_attn[:, cs]).astype(bf16),
            "wk": np.ascontiguousarray(w_attn[:, csC]).astype(bf16),
            "wv": np.ascontiguousarray(w_attn[:, cs2C]).astype(bf16),
            "wp": np.ascontiguousarray(w_proj[cs, :]).astype(bf16),
            "bps": np.ascontiguousarray(np.concatenate([
                b_attn[cs].reshape(2, P).T,
                b_attn[csC].reshape(2, P).T,
                (b_attn[cs2C] + SHIFT).reshape(2, P).T,
                p_param[cs].reshape(2, P).T,
            ], axis=1).astype(np.float32)),
            "cst": cst,
            "onesr": onesr,
        })
    return in_maps


def kernel(x, w_attn, b_attn, w_proj, b_proj, p_param, _trace=False):
    x = np.asarray(x, dtype=np.float32)
    w_attn = np.asarray(w_attn, dtype=np.float32)
    b_attn = np.asarray(b_attn, dtype=np.float32)
    w_proj = np.asarray(w_proj, dtype=np.float32)
    b_proj = np.asarray(b_proj, dtype=np.float32)
    p_param = np.asarray(p_param, dtype=np.float32)

    # p == 1 admits a cheaper final transform (no per-tile ln/exp)
    p_eff = np.sign(np.sign(p_param) + 0.5) * np.clip(np.abs(p_param),
                                                      P_MIN, P_MAX)
    fast_p1 = bool(np.all(p_eff == 1.0))

    key = ("nc", fast_p1)
    if key not in _CACHE:
        _CACHE[key] = _build(fast_p1)
    nc = _CACHE[key]

    in_maps = _host_inputs(x, w_attn, b_attn, w_proj, p_param)
    res = run_bass_kernel_spmd(nc, in_maps, core_ids=list(range(8)),
                               trace=_trace)
    _CACHE["last_result"] = res

    out = np.zeros((B, T, C), dtype=np.float32)
    for core in range(8):
        b = core // 4
        out[b] += res.results[core]["out_p"]
    out += b_proj[None, None, :]
    return out


if __name__ == "__main__":
    rng = np.random.default_rng(0)
    ins = {
        "x": rng.standard_normal((B, T, C), dtype=np.float32),
        "w_attn": (rng.standard_normal((C, 3 * C), dtype=np.float32) * 0.02),
        "b_attn": np.zeros(3 * C, np.float32),
        "w_proj": (rng.standard_normal((C, C), dtype=np.float32) * 0.02),
        "b_proj": np.zeros(C, np.float32),
        "p_param": np.ones(C, np.float32),
    }
    out = kernel(**ins)
    print("ran, out shape", out.shape, "finite:", np.isfinite(out).all())


# revision 23
# speedup vs baseline: 1.2345x; 1.0130x over previous
"""Trainium2 Bass kernel for nn_CausalGemAttention.

Reference computation (B=2, T=2048, C=1024, H=16, d=64):
    qkv = x @ w_attn + b_attn ; q,k,v = split(qkv)
    p = sign(sign(p_param)+0.5) * clamp(|p_param|, 1e-4, 1e3)
    vc = clip(|v + 5|, 1e-10); z = p*ln(vc); zmax = max_T(z); v' = exp(z - zmax)
    att = causal_softmax(q k^T / sqrt(d)); mean = att @ v'
    y = exp((zmax + ln(mean)) / p) - 5 ; out = y @ w_proj + b_proj

Sharding: 8 cores = 2 (batch) x 4 (head groups of 4 heads / 256 channels).
Each core computes qkv for its head group (contraction over full C), local
attention, and a partial projection (w_proj rows of its channels); host sums
the 4 partials per batch and adds b_proj.

Matmul operands are bf16 with fp32 PSUM accumulation.  To keep bf16 rounding
out of the softmax average (the +5 shift amplifies relative error of the
mean ~8x), v' is centered per channel before the PV matmul:
    mean = num''/den + cmid   with   v'' = v' - cmid
cmid is carried in fp32 and re-added exactly.  When p == 1 (the shipped
configuration) the final transform reduces to y = exp(zmax)*mean - 5 and is
computed without any per-tile ln/exp; a general-p fallback path exists.

Performance notes (per core, ~197us on HW):
  - flash-attention-free layout: scores computed as S^T = K^T.T Q^T with
    keys on PSUM partitions, so softmax denominators come from a ones
    column appended to v'' in the PV matmul (out = [v''|1].T @ P).
  - q/k are zero-padded to K=128 contraction (K=64 matmuls measure ~1.5x
    slower per N-cycle on the PE).
  - softmax exp runs on ScalarE over [128,1024] PSUM pairs to amortize the
    ~352-cycle ACT op overhead; Ln/Exp stay in one ACT table set.
  - the 1/den reciprocal uses reciprocal_approx_fast + a K=1 outer-product
    broadcast on the PE; post-chains are emitted one q-block late so the PE
    never stalls on the DVE chain.
  - causal sub-ranges: diagonal-band tiles only compute/exp/accumulate the
    valid column range.
  - projection is emitted interleaved with the tail of attention.
"""

import sys
sys.path.insert(0, "/opt/trn_rl_repo")

import numpy as np
import ml_dtypes

import concourse.bacc as bacc
import concourse.tile as tile
from concourse import mybir
from concourse.bass_utils import run_bass_kernel_spmd

F32 = mybir.dt.float32
F32R = mybir.dt.float32r
BF16 = mybir.dt.bfloat16
AF = mybir.ActivationFunctionType
ALU = mybir.AluOpType
AX = mybir.AxisListType

B, T, C, H, D = 2, 2048, 1024, 16, 64
P = 128
CL = 256            # channels per core (4 heads x 64)
KC = C // P         # 8 contraction chunks for qkv
NQ = T // 512       # 4 query blocks of 512
NK = T // P         # 16 key tiles of 128
SHIFT = 5.0
P_MIN, P_MAX, V_MIN = 1e-4, 1e3, 1e-10
SM_SCALE = 1.0 / 8.0  # 1/sqrt(64)

_CACHE = {}


def _build(fast_p1):
    nc = bacc.Bacc("TRN2", target_bir_lowering=False, debug=False)

    xt_d = nc.dram_tensor("xt", [C, T], BF16, kind="ExternalInput")
    wq_d = nc.dram_tensor("wq", [C, CL], BF16, kind="ExternalInput")
    wk_d = nc.dram_tensor("wk", [C, CL], BF16, kind="ExternalInput")
    wv_d = nc.dram_tensor("wv", [C, CL], BF16, kind="ExternalInput")
    wp_d = nc.dram_tensor("wp", [CL, C], BF16, kind="ExternalInput")
    # bps = [bq(2) | bk(2) | bv5(2) | pp(2)] per chunk, fp32
    bps_d = nc.dram_tensor("bps", [P, 8], F32, kind="ExternalInput")
    # cst = [masks(4x512) | ident(64) | onesc(16)] packed along free dim
    cst_d = nc.dram_tensor("cst", [P, 2128], BF16, kind="ExternalInput")
    or_d = nc.dram_tensor("onesr", [1, 64], F32R, kind="ExternalInput")
    out_d = nc.dram_tensor("out_p", [T, C], F32, kind="ExternalOutput")

    with tile.TileContext(nc) as tc:
        with (
            tc.tile_pool(name="consts", bufs=1) as cp,
            tc.tile_pool(name="qk", bufs=1) as qkp,
            tc.tile_pool(name="vy", bufs=1) as vyp,
        ):
            cst = cp.tile([P, 2128], BF16)
            onesr = cp.tile([1, 64], F32R)
            bps = cp.tile([P, 8], F32)
            nc.sync.dma_start(bps[:], bps_d[:])
            ident = cst[:, 2048:2112]
            bq_sb = bps[:, 0:2]
            bk_sb = bps[:, 2:4]
            bv5_sb = bps[:, 4:6]
            pp_sb = bps[:, 6:8]

            # p = sign(sign(pp)+0.5) * clamp(|pp|, P_MIN, P_MAX); ip = 1/p
            sgn = cp.tile([P, 2], F32)
            ab = cp.tile([P, 2], F32)
            p_sb = cp.tile([P, 2], F32)
            # allcp packs [ip | zmaxp | cmid | ezp | ecp5] x 2 chunks
            allcp = cp.tile([P, 5, 2], F32)
            ip_sb = allcp[:, 0, :]
            zmaxp = allcp[:, 1, :]
            cmid = allcp[:, 2, :]
            ezp = allcp[:, 3, :]
            ecp5 = allcp[:, 4, :]
            nc.scalar.activation(sgn[:], pp_sb[:], AF.Sign)
            nc.vector.tensor_scalar_add(sgn[:], sgn[:], 0.5)
            nc.scalar.activation(sgn[:], sgn[:], AF.Sign)
            nc.scalar.activation(ab[:], pp_sb[:], AF.Abs)
            nc.vector.tensor_scalar(ab[:], ab[:], float(P_MIN), float(P_MAX),
                                    ALU.max, ALU.min)
            nc.vector.tensor_tensor(p_sb[:], sgn[:], ab[:], ALU.mult)
            nc.vector.reciprocal(ip_sb[:], p_sb[:])

            negzmax = cp.tile([P, 2], F32)
            zmin_sb = cp.tile([P, 2], F32)
            # allh: per-head [64,1] base-0 views of allcp, [64, const, head]
            allh = cp.tile([64, 5, 4], F32)
            iph = allh[:, 0, :]
            zmh = allh[:, 1, :]
            cmh = allh[:, 2, :]
            eph = allh[:, 3, :]
            ech = allh[:, 4, :]

            qT = qkp.tile([P, 2, T], BF16)   # q^T: [c%128, c//128, t]
            kT = qkp.tile([P, 2, T], BF16)
            vnat = vyp.tile([P, 4, NK, 65], BF16)  # [tk%128, head, tk//128, d|1]
            yT = vyp.tile([P, 2, T], BF16)
            wp_sb = vyp.tile([P, 2, C], BF16)
            # padded per-head q/k (K=128 with zero rows 64:127): K=64 matmuls
            # measure ~1.5x slower than K=128 on the PE, so pad instead.
            qTp = vyp.tile([P, 4, T], BF16)
            kTp = vyp.tile([P, 4, T], BF16)

            # ---------------- Phase A: qkv;  B: v transform + transposes -----
            with (
                tc.tile_pool(name="pA", bufs=1) as pA,
                tc.tile_pool(name="pB", bufs=1) as pB,
                tc.tile_pool(name="psA", bufs=6, space="PSUM") as psA,
            ):
                xt_sb = pA.tile([P, KC, T], BF16)
                wq_sb = pA.tile([P, KC, CL], BF16)
                wk_sb = pA.tile([P, KC, CL], BF16)
                wv_sb = pA.tile([P, KC, CL], BF16)
                nc.sync.dma_start(wv_sb[:], wv_d[:].rearrange("(a p) m -> p a m", p=P))
                nc.sync.dma_start(wq_sb[:], wq_d[:].rearrange("(a p) m -> p a m", p=P))
                nc.sync.dma_start(wk_sb[:], wk_d[:].rearrange("(a p) m -> p a m", p=P))
                for kc in range(KC):
                    for hf in range(2):
                        nc.sync.dma_start(
                            xt_sb[:, kc, hf * 1024:(hf + 1) * 1024],
                            xt_d[kc * P:(kc + 1) * P, hf * 1024:(hf + 1) * 1024])
                nc.sync.dma_start(cst[:], cst_d[:])
                nc.sync.dma_start(onesr[:], or_d[:])
                nc.sync.dma_start(wp_sb[:],
                                  wp_d[:].rearrange("(c p) n -> p c n", p=P))
                nc.gpsimd.memset(qTp[64:128, :, :], 0.0)
                nc.gpsimd.memset(kTp[64:128, :, :], 0.0)

                vT = pB.tile([P, 2, T], F32)
                vpT = pB.tile([P, 2, T], BF16)

                def qkv_group(wsb, kind, m, nt):
                    ps = psA.tile([P, 512], F32, tag="ev", name="ev")
                    for kc in range(KC):
                        nc.tensor.matmul(
                            ps[:],
                            wsb[:, kc, m * P:(m + 1) * P],
                            xt_sb[:, kc, nt * 512:(nt + 1) * 512],
                            start=(kc == 0), stop=(kc == KC - 1),
                        )
                    tsl = slice(nt * 512, (nt + 1) * 512)
                    if kind == "q":
                        nc.vector.tensor_scalar_add(
                            qT[:, m, tsl], ps[:], bq_sb[:, m:m + 1])
                    elif kind == "k":
                        nc.vector.tensor_scalar_add(
                            kT[:, m, tsl], ps[:], bk_sb[:, m:m + 1])
                    else:
                        # |v + b + SHIFT| directly out of PSUM
                        nc.scalar.activation(
                            vT[:, m, tsl], ps[:], AF.Abs,
                            bias=bv5_sb[:, m:m + 1])

                # v first so its transform overlaps the q/k matmuls
                for m in range(2):
                    for nt in range(NQ):
                        qkv_group(wv_sb, "v", m, nt)

                # transform: z = p*ln(clip(vc)); zmax/zmin; v'' = e^(z-zmax)-cmid
                for m in range(2):
                    nc.vector.tensor_scalar_max(vT[:, m, :], vT[:, m, :],
                                                float(V_MIN))
                    nc.scalar.activation(vT[:, m, :], vT[:, m, :], AF.Ln)
                for m in range(2):
                    nc.vector.tensor_scalar_mul(vT[:, m, :], vT[:, m, :],
                                                p_sb[:, m:m + 1])
                    nc.vector.tensor_reduce(negzmax[:, m:m + 1], vT[:, m, :], AX.X,
                                            op=ALU.max, negate=True)
                    nc.vector.tensor_reduce(zmin_sb[:, m:m + 1], vT[:, m, :], AX.X,
                                            op=ALU.min)
                    nc.vector.scalar_tensor_tensor(
                        zmaxp[:, m:m + 1], negzmax[:, m:m + 1], -1.0,
                        ip_sb[:, m:m + 1], ALU.mult, ALU.mult)
                for m in range(2):
                    # cmid = 0.5*(1 + exp(zmin - zmax))
                    nc.scalar.activation(cmid[:, m:m + 1], zmin_sb[:, m:m + 1],
                                         AF.Exp, bias=negzmax[:, m:m + 1])
                    nc.vector.tensor_scalar(cmid[:, m:m + 1], cmid[:, m:m + 1],
                                            1.0, 0.5, ALU.add, ALU.mult)
                    # ezp = exp(zmax); ecp5 = ezp*cmid - 5
                    nc.scalar.activation(ezp[:, m:m + 1], negzmax[:, m:m + 1],
                                         AF.Exp, scale=-1.0)
                    nc.vector.scalar_tensor_tensor(
                        ecp5[:, m:m + 1], ezp[:, m:m + 1], 0.0,
                        cmid[:, m:m + 1], ALU.bypass, ALU.mult)
                    nc.vector.tensor_scalar_add(ecp5[:, m:m + 1], ecp5[:, m:m + 1],
                                                -SHIFT)
                    # v' (fp32, in place over z) then centered bf16 copy
                    nc.scalar.activation(vT[:, m, :], vT[:, m, :], AF.Exp,
                                         bias=negzmax[:, m:m + 1])
                    nc.vector.tensor_scalar_sub(vpT[:, m, :], vT[:, m, :],
                                                cmid[:, m:m + 1])

                # q/k matmuls (PE work that overlaps the v transform above)
                for m in range(2):
                    for nt in range(NQ):
                        qkv_group(wq_sb, "q", m, nt)
                        qkv_group(wk_sb, "k", m, nt)
                    for h in (2 * m, 2 * m + 1):
                        base = 64 * (h % 2)
                        nc.gpsimd.dma_start(qTp[0:64, h, :],
                                            qT[base:base + 64, m, :])
                        nc.gpsimd.dma_start(kTp[0:64, h, :],
                                            kT[base:base + 64, m, :])

                # per-head constants at partition base 0: heads (0,2) from
                # chunk rows 0:64, heads (1,3) from rows 64:128
                nc.sync.dma_start(allh[:, :, 0::2], allcp[0:64, :, :])
                nc.sync.dma_start(allh[:, :, 1::2], allcp[64:128, :, :])
                for h in range(4):
                    nc.sync.dma_start(vnat[:, h, :, 64], cst_d[:, 2112:2128])

                # transpose v''^T [d, tk] -> vnat [tk, d], 8 k-tiles per bank
                for h in range(4):
                    base, ch = 64 * (h % 2), h // 2
                    for half in range(2):
                        trp = psA.tile([P, 512], BF16, tag="tr", name="trp", bufs=2)
                        for j in range(8):
                            kt = half * 8 + j
                            nc.tensor.transpose(
                                trp[:, j * 64:(j + 1) * 64],
                                vpT[base:base + 64, ch, kt * P:(kt + 1) * P],
                                cst[base:base + 64, 2048:2112],
                            )
                        nc.vector.tensor_copy(
                            vnat[:, h, half * 8:(half + 1) * 8, 0:64],
                            trp[:].rearrange("p (a b) -> p a b", a=8),
                        )

            # ---------------- Phase C: attention ------------------------------
            with (
                tc.tile_pool(name="att", bufs=5) as att,
                tc.tile_pool(name="small", bufs=3) as sm,
                tc.tile_pool(name="outp", bufs=3) as op_,
                tc.tile_pool(name="psS", bufs=2, space="PSUM") as psS,
                tc.tile_pool(name="psV", bufs=2, space="PSUM") as psV,
                tc.tile_pool(name="psX", bufs=2, space="PSUM") as psX,
            ):
                pending = []   # deferred post-chains (emitted mid next q-block)

                def proj_group(tq):
                    po = op_.tile([P, C], F32, tag="po", name="po")
                    for nh in range(2):
                        pj = psX.tile([P, 512], F32, tag="x", name="pj")
                        for c in range(2):
                            nc.tensor.matmul(
                                pj[:],
                                yT[:, c, tq * P:(tq + 1) * P],
                                wp_sb[:, c, nh * 512:(nh + 1) * 512],
                                start=(c == 0), stop=(c == 1),
                            )
                        if nh == 0:
                            nc.scalar.activation(po[:, 0:512], pj[:], AF.Copy)
                        else:
                            nc.vector.tensor_copy(po[:, 512:1024], pj[:])
                    nc.gpsimd.dma_start(out_d[tq * P:(tq + 1) * P, :], po[:])

                def post_chain(pv_t, h):
                    # mean'' = num''/den ; y = ezp*mean'' + (ezp*cmid - 5)
                    dcp = sm.tile([1, 512], F32, tag="dcp", name="dcp")
                    nc.vector.tensor_copy(dcp[:], pv_t[64:65, :])
                    rdf = sm.tile([1, 512], F32, tag="rdf", name="rdf")
                    nc.vector.reciprocal_approx_fast(rdf[:], dcp[:])
                    rd = sm.tile([1, 512], F32R, tag="rd", name="rd")
                    nc.vector.tensor_copy(rd[:], rdf[:])
                    bc = psX.tile([64, 512], F32, tag="x", name="bc",
                                  padded_shape=[P, 512])
                    nc.tensor.matmul(bc[:], onesr[:], rd[:], start=True, stop=True)
                    me = sm.tile([64, 512], F32, tag="me", name="me")
                    nc.vector.tensor_copy(me[:], pv_t[0:64, :])
                    yh = sm.tile([64, 512], BF16, tag="yh", name="yh")
                    if fast_p1:
                        y1 = sm.tile([64, 512], F32, tag="y1", name="y1")
                        nc.vector.scalar_tensor_tensor(
                            y1[:], me[:], eph[:, h:h + 1], bc[:],
                            ALU.mult, ALU.mult)
                        nc.vector.tensor_scalar_add(yh[:], y1[:],
                                                    ech[:, h:h + 1])
                    else:
                        nc.vector.tensor_tensor(me[:], me[:], bc[:], ALU.mult)
                        nc.vector.tensor_scalar_add(me[:], me[:],
                                                    cmh[:, h:h + 1])
                        nc.scalar.activation(me[:], me[:], AF.Ln)
                        nc.scalar.activation(yh[:], me[:], AF.Exp,
                                             scale=iph[:, h:h + 1],
                                             bias=zmh[:, h:h + 1])
                        nc.vector.tensor_scalar_add(yh[:], yh[:], -SHIFT)
                    base, ch = 64 * (h % 2), h // 2
                    qi_ = post_chain_qi[0]
                    nc.gpsimd.dma_start(
                        yT[base:base + 64, ch, qi_ * 512:(qi_ + 1) * 512], yh[:])

                post_chain_qi = [0]

                for hp in range(2):
                    h0, h1 = 2 * hp, 2 * hp + 1
                    ch = hp
                    for qi in range(NQ):
                        npair = 2 * (qi + 1)       # kt pairs (kt = 2a, 2a+1)
                        qsl = slice(qi * 512, (qi + 1) * 512)
                        pv = [psV.tile([65, 512], F32, tag="pv", name=f"pv{_i}")
                              for _i in range(2)]
                        prev = None
                        for a in range(npair):
                            ptile = []
                            s_ps = [psS.tile([P, 1024], F32, tag="s",
                                             name=f"s{_i}") for _i in range(2)]
                            j0 = 2 * a - 4 * qi
                            # causal column offset: tile kt only has valid
                            # scores for tq >= 128*j (j = kt - 4*qi)
                            off0 = P * max(j0, 0)
                            for half in range(2):
                                kt = 2 * a + half
                                off = P * max(kt - 4 * qi, 0)
                                ksl = slice(kt * P, (kt + 1) * P)
                                qsub = slice(qi * 512 + off, (qi + 1) * 512)
                                for i, h in enumerate((h0, h1)):
                                    nc.tensor.matmul(
                                        s_ps[i][:, half * 512 + off:
                                                (half + 1) * 512],
                                        kTp[:, h, ksl],
                                        qTp[:, h, qsub],
                                        start=True, stop=True,
                                        skip_group_check=True,
                                    )
                            for i in range(2):
                                pt = att.tile([P, 1024], BF16, tag="pT",
                                              name="pt")
                                nc.scalar.activation(pt[:, off0:1024],
                                                     s_ps[i][:, off0:1024],
                                                     AF.Exp, scale=SM_SCALE)
                                if j0 >= 0:   # diagonal band: mask pair
                                    nc.vector.tensor_mul(
                                        pt[:, off0:1024],
                                        pt[:, off0:1024],
                                        cst[:, j0 * 512 + off0:
                                            (j0 + 2) * 512])
                                ptile.append(pt)
                            if prev is not None:
                                pa_, pp0, pp1 = prev
                                for i, ppt in enumerate((pp0, pp1)):
                                    for half in range(2):
                                        kt = 2 * pa_ + half
                                        o_ = P * max(kt - 4 * qi, 0)
                                        nc.tensor.matmul(
                                            pv[i][:, o_:512],
                                            vnat[:, (h0, h1)[i], kt, :],
                                            ppt[:, half * 512 + o_:
                                                (half + 1) * 512],
                                            start=(kt == 0), stop=False,
                                            skip_group_check=True,
                                        )
                            if a == 1 and pending:
                                for fn in pending:
                                    fn()
                                pending.clear()
                                if hp == 1 and qi > 0:
                                    for tq in range(4 * (qi - 1), 4 * qi):
                                        proj_group(tq)
                            prev = (a, ptile[0], ptile[1])
                        pa_, pp0, pp1 = prev
                        for i, ppt in enumerate((pp0, pp1)):
                            for half in range(2):
                                kt = 2 * pa_ + half
                                o_ = P * max(kt - 4 * qi, 0)
                                nc.tensor.matmul(
                                    pv[i][:, o_:512],
                                    vnat[:, (h0, h1)[i], kt, :],
                                    ppt[:, half * 512 + o_:(half + 1) * 512],
                                    start=(kt == 0), stop=(half == 1),
                                    skip_group_check=True,
                                )

                        def mk(pv_t, h, qi):
                            def fn():
                                post_chain_qi[0] = qi
                                post_chain(pv_t, h)
                            return fn
                        pending.append(mk(pv[0], h0, qi))
                        pending.append(mk(pv[1], h1, qi))
                for fn in pending:
                    fn()
                pending.clear()
                for tq in range(12, 16):
                    proj_group(tq)


    nc.finalize()
    return nc


def _host_inputs(x, w_attn, b_attn, w_proj, p_param):
    """Build the 8 per-core input dicts."""
    bf16 = ml_dtypes.bfloat16
    ident = np.concatenate([np.eye(64, dtype=np.float32)] * 2, axis=0)
    xx = np.arange(P, dtype=np.int64)[:, None]
    yy = np.arange(512, dtype=np.int64)[None, :]
    masks = np.concatenate(
        [(yy - xx - P * j >= 0).astype(np.float32) for j in range(4)], axis=1)
    onesc = np.ones((P, NK), dtype=np.float32)
    cst = np.concatenate([masks, ident, onesc], axis=1).astype(bf16)
    onesr = np.ones((1, 64), dtype=np.float32)

    xts = [np.ascontiguousarray(x[b].T).astype(bf16) for b in range(B)]
    in_maps = []
    for core in range(8):
        b, hg = divmod(core, 4)
        cs = slice(hg * CL, (hg + 1) * CL)
        csC = slice(C + hg * CL, C + (hg + 1) * CL)
        cs2C = slice(2 * C + hg * CL, 2 * C + (hg + 1) * CL)
        in_maps.append({
            "xt": xts[b],
            "wq": np.ascontiguousarray(w_attn[:, cs]).astype(bf16),
            "wk": np.ascontiguousarray(w_attn[:, csC]).astype(bf16),
            "wv": np.ascontiguousarray(w_attn[:, cs2C]).astype(bf16),
            "wp": np.ascontiguousarray(w_proj[cs, :]).astype(bf16),
            "bps": np.ascontiguousarray(np.concatenate([
                b_attn[cs].reshape(2, P).T,
                b_attn[csC].reshape(2, P).T,
                (b_attn[cs2C] + SHIFT).reshape(2, P).T,
                p_param[cs].reshape(2, P).T,
            ], axis=1).astype(np.float32)),
            "cst": cst,
            "onesr": onesr,
        })
    return in_maps


def kernel(x, w_attn, b_attn, w_proj, b_proj, p_param, _trace=False):
    x = np.asarray(x, dtype=np.float32)
    w_attn = np.asarray(w_attn, dtype=np.float32)
    b_attn = np.asarray(b_attn, dtype=np.float32)
    w_proj = np.asarray(w_proj, dtype=np.float32)
    b_proj = np.asarray(b_proj, dtype=np.float32)
    p_param = np.asarray(p_param, dtype=np.float32)

    # p == 1 admits a cheaper final transform (no per-tile ln/exp)
    p_eff = np.sign(np.sign(p_param) + 0.5) * np.clip(np.abs(p_param),
                                                      P_MIN, P_MAX)
    fast_p1 = bool(np.all(p_eff == 1.0))

    key = ("nc", fast_p1)
    if key not in _CACHE:
        _CACHE[key] = _build(fast_p1)
    nc = _CACHE[key]

    in_maps = _host_inputs(x, w_attn, b_attn, w_proj, p_param)
    res = run_bass_kernel_spmd(nc, in_maps, core_ids=list(range(8)),
                               trace=_trace)
    _CACHE["last_result"] = res

    out = np.zeros((B, T, C), dtype=np.float32)
    for core in range(8):
        b = core // 4
        out[b] += res.results[core]["out_p"]
    out += b_proj[None, None, :]
    return out


if __name__ == "__main__":
    rng = np.random.default_rng(0)
    ins = {
        "x": rng.standard_normal((B, T, C), dtype=np.float32),
        "w_attn": (rng.standard_normal((C, 3 * C), dtype=np.float32) * 0.02),
        "b_attn": np.zeros(3 * C, np.float32),
        "w_proj": (rng.standard_normal((C, C), dtype=np.float32) * 0.02),
        "b_proj": np.zeros(C, np.float32),
        "p_param": np.ones(C, np.float32),
    }
    out = kernel(**ins)
    print("ran, out shape", out.shape, "finite:", np.isfinite(out).all())
